# revision 4
# baseline (speedup 1.0000x reference)
"""Trainium2 Bass kernel for nn_BidirectionalGRU (B=8,S=1024,D=1024), v2.

Sharding: 8 cores = 8 time-chunks of L=128 tokens. Every core runs BOTH
GRU directions (streams A=fwd, B=bwd) over its own chunk window with
warm-up margins, so all cross-chunk dependence is absorbed by warm-up
(GRU state contracts; W=32 gives ~1e-7 end-to-end error on CPU) and no
collectives are needed. Per-core differences are input data only
(x windows, z-gate pad masks); the program is SPMD-uniform.

Windows (chunk c, warm-up W, lo = 128c-2W):
  x window:   t in [lo, lo+T0X), T0X=L+4W, stored ascending.
  L0 scan:    A ascends window idx from 0; B descends from T0X-1;
              T0=L+3W steps; h0 flushed for steps>=W into hT0[D,T0H,B]
              stored ascending-t (B's flush reversed), T0H=L+2W,
              covering t in [lo+W, lo+W+T0H).
  xg1:        stream A tokens = h0 idx [0,T1) asc; B = idx [T0H-1..W]
              desc (= its scan order); T1=L+W.
  L1 scan:    both fetch xg1 ascending (already scan-ordered); flush
              steps>=W into hT1[D,L,B] ascending-t (B reversed).
  Out-of-range steps (edge cores) are neutralized by zpad=+40 on the
  z-gate pre-activation (z=1 freezes h=0 exactly); host pads x with 0.
"""
import contextlib
import numpy as np

import concourse.bacc as bacc
import concourse.tile as tile
from concourse import mybir
from concourse.bass import ds
from concourse.bass_utils import run_bass_kernel_spmd
from concourse.masks import make_identity

F32 = mybir.dt.float32
F32R = mybir.dt.float32r
BF16 = mybir.dt.bfloat16
AF = mybir.ActivationFunctionType
ALU = mybir.AluOpType

B, S, D, H3, G, FFN = 8, 1024, 1024, 3072, 4, 2816
NCORE = 8
L = S // NCORE               # 128 owned tokens per core
W = 32                       # warm-up steps
T0X = L + 4 * W              # x window tokens (256)
T0 = L + 3 * W               # L0 scan steps (224)
T0H = L + 2 * W              # h0 stored tokens (192)
T1 = L + W                   # L1 scan steps (160)
KD = D // 128                # 8
KF = FFN // 128              # 22
EPS = 1e-5
NP = 104                     # partitions spanned by grouped layout


# ================================================================ host prep
def gate_perm():
    idx = []
    for j in range(G):
        for blk in range(3):
            base = blk * 1024 + j * 256
            idx.extend(range(base, base + 256))
    return np.array(idx)

PERM = gate_perm()


def prep_scan_weights(w_hh_d):
    wp = w_hh_d[PERM]
    wt = wp.T.reshape(KD, 128, H3).transpose(1, 0, 2)
    return np.ascontiguousarray(wt.reshape(128, KD * H3), dtype=np.float32)


def prep_gemm_weights(w_ih_d, norm_w=None):
    wp = w_ih_d[PERM]
    if norm_w is not None:
        wp = wp * norm_w[None, :]
    return np.ascontiguousarray(wp.T, dtype=np.float32)


def prep_gemm_bias(b_ih_d, b_hh_d):
    bi = b_ih_d[PERM].copy()
    bh = b_hh_d[PERM]
    m = np.where(np.arange(H3) % 768 < 512, bh, 0.0)
    b = (bi + m).astype(np.float32)
    return np.ascontiguousarray(np.broadcast_to(b, (128, H3)), dtype=np.float32)


def prep_bhn_scan(b_hh_d):
    bh = b_hh_d[PERM].reshape(G, 3, 256)[:, 2, :]
    out = np.zeros((128, 256), np.float32)
    for j in range(G):
        out[32 * j:32 * j + 32, :] = bh[j][None, :]
    return out


# ============================================================ device builders
def build_norm_stats(tc, x_nat, s_sb, nt):
    nc = tc.nc
    with tc.tile_pool(name="nstat", bufs=3) as pool:
        for i in range(nt):
            xt = pool.tile([128, D], F32, name="xt")
            nc.sync.dma_start(xt[:], x_nat[i * 128:(i + 1) * 128, :])
            sq = pool.tile([128, D], F32, name="sq")
            ss = pool.tile([128, 1], F32, name="ss")
            nc.scalar.activation(sq[:], xt[:], AF.Square, accum_out=ss[:])
            m = pool.tile([128, 1], F32, name="m")
            nc.vector.tensor_scalar(m[:], ss[:], 1.0 / D, EPS,
                                    op0=ALU.mult, op1=ALU.add)
            r = pool.tile([128, 1], F32, name="r")
            nc.vector.reciprocal(r[:], m[:])
            nc.scalar.activation(s_sb[:, i:i + 1], r[:], AF.Sqrt)


def build_xg_gemm(tc, stat_views, n_k, ws, biases, s_sb, out_vs,
                  zeros_st, zrhs, nt, wdt=F32R, sdt=F32R):
    """out[token, g, 768c] = s*(x @ w) + bias for 1-2 streams sharing
    stationary token tiles. stat_views: n_k APs [128, nt*128]."""
    nc = tc.nc
    ns = len(ws)
    U = 4
    while nt % U:
        U //= 2
    with contextlib.ExitStack() as c:
        wp = c.enter_context(tc.tile_pool(name="xg_w", bufs=1))
        pool = c.enter_context(tc.tile_pool(name="xg_t", bufs=3))
        stp = c.enter_context(tc.tile_pool(name="xg_s", bufs=2))
        pp = c.enter_context(tc.tile_pool(name="xg_p", bufs=4, space="PSUM"))

        bias_sb = wp.tile([128, ns * H3], F32, name="bias_sb")
        for si in range(ns):
            nc.sync.dma_start(bias_sb[:, si * H3:(si + 1) * H3],
                              biases[si][:, :])
        for c0 in range(0, H3, 512):
            wc = pool.tile([128, ns * n_k * 512], wdt, name="wc")
            for si in range(ns):
                for k in range(n_k):
                    nc.sync.dma_start(
                        wc[:, (si * n_k + k) * 512:(si * n_k + k + 1) * 512],
                        ws[si][k * 128:(k + 1) * 128, c0:c0 + 512])
            with tc.For_i(0, nt // U) as iv:
                for u in range(U):
                    tv = iv * U + u
                    tok = tv * 128
                    sts = []
                    for k in range(n_k):
                        stt = stp.tile([128, 128], sdt, name=f"st{k}")
                        nc.sync.dma_start(stt[:],
                                          stat_views[k][:, ds(tok, 128)])
                        sts.append(stt)
                    for si in range(ns):
                        ps = pp.tile([128, 512], F32, name="ps")
                        nc.tensor.matmul(ps[:], zeros_st[:], zrhs[:],
                                         start=True, stop=False)
                        for k in range(n_k):
                            nc.tensor.matmul(
                                ps[:], sts[k][:],
                                wc[:, (si * n_k + k) * 512:
                                   (si * n_k + k + 1) * 512],
                                start=False, stop=(k == n_k - 1))
                        o = pool.tile([128, 512], F32, name="o")
                        if s_sb is not None:
                            nc.vector.scalar_tensor_tensor(
                                o[:], ps[:], s_sb[:, ds(tv, 1)],
                                bias_sb[:, si * H3 + c0:si * H3 + c0 + 512],
                                op0=ALU.mult, op1=ALU.add)
                        else:
                            nc.vector.tensor_add(
                                o[:], ps[:],
                                bias_sb[:, si * H3 + c0:si * H3 + c0 + 512])
                        cc = c0
                        while cc < c0 + 512:
                            g, gc = divmod(cc, 768)
                            take = min(768 - gc, c0 + 512 - cc)
                            nc.sync.dma_start(
                                out_vs[si][ds(tok, 128), g, gc:gc + take],
                                o[:, cc - c0:cc - c0 + take])
                            cc += take


class ScanStream:
    """State for one of two interleaved GRU scan directions.

    rev_base: None -> xg fetched at storage idx (off + iv*U + u);
              int  -> fetched at (rev_base - (off + iv*U + u)).
    flush_rev: owned h stored descending into hT_out's t axis.
    """

    def __init__(self, tc, name, ctx, w_src, bhn_src, zpad_src, xg_v,
                 hT_out, rev_base, flush_rev, n_steps, flush_lo, zeros_bf,
                 U=16):
        nc = tc.nc
        self.tc = tc
        self.name = name
        self.rev_base = rev_base
        self.flush_rev = flush_rev
        self.n_steps = n_steps
        self.flush_lo = flush_lo
        self.hT_out = hT_out            # [D, n_out, B], ascending t
        self.n_out = hT_out.shape[1]
        self.U = U
        wp = ctx.enter_context(tc.tile_pool(name=f"w_{name}", bufs=1))
        st = ctx.enter_context(tc.tile_pool(name=f"s_{name}", bufs=1))
        self.pool = ctx.enter_context(tc.tile_pool(name=f"t_{name}", bufs=3))
        self.pp = ctx.enter_context(
            tc.tile_pool(name=f"p_{name}", bufs=1, space="PSUM"))
        self.ppt = ctx.enter_context(
            tc.tile_pool(name=f"pt_{name}", bufs=1, space="PSUM"))

        self.w_sb = wp.tile([128, KD * H3], BF16, name="w_sb")
        nc.sync.dma_start(self.w_sb[:], w_src[:, :])
        self.bhn = wp.tile([128, 256], F32, name="bhn")
        nc.sync.dma_start(self.bhn[:], bhn_src[:, :])
        self.zpad = wp.tile([128, n_steps], F32, name="zpad")
        nc.sync.dma_start(self.zpad[:], zpad_src[:, 0:n_steps])

        self.hgrp = st.tile([128, 256], F32, name="hgrp")
        nc.gpsimd.memset(self.hgrp[:], 0.0)
        self.hT_hist = st.tile([128, U * 64], BF16, name="hT_hist")
        nc.sync.dma_start(self.hT_hist[:], zeros_bf[:, 0:U * 64])
        self.xg_t = xg_v.rearrange("(t b) g c -> t g b c", b=B)

    def step(self, iv, u, off, zeros_st, zrhs, ident):
        nc = self.tc.nc
        pool, pp, ppt = self.pool, self.pp, self.ppt
        U = self.U
        slot, pslot = u, (u - 1) % U
        if self.rev_base is None:
            t_el = iv * U + (u + off)
        else:
            t_el = iv * (-U) + (self.rev_base - u - off)
        xgt = pool.tile([128, 768], F32, name="xgt")
        for j in range(G):
            srcj = self.xg_t[ds(t_el, 1), j, :, :].rearrange(
                "a b c -> (a b) c")
            nc.sync.dma_start(xgt[32 * j:32 * j + B, :], srcj)

        gates = pp.tile([128, 768], F32, name="gates")
        nc.tensor.matmul(gates[:, 0:512], zeros_st[:], zrhs[:],
                         start=True, stop=False)
        nc.tensor.matmul(gates[:, 512:768], zeros_st[:], zrhs[:, 0:256],
                         start=True, stop=False)
        for k in range(KD):
            j2, c2 = divmod(k, 2)
            lof = pslot * 64 + c2 * 32 + j2 * 8
            lhsT = self.hT_hist[:, lof:lof + 8]
            for j in range(G):
                wof = k * H3 + j * 768
                nc.tensor.matmul(gates[32 * j:32 * j + 8, 0:512], lhsT,
                                 self.w_sb[:, wof:wof + 512],
                                 start=False, stop=False,
                                 tile_position=(0, 32 * j))
                nc.tensor.matmul(gates[32 * j:32 * j + 8, 512:768], lhsT,
                                 self.w_sb[:, wof + 512:wof + 768],
                                 start=False, stop=(k == KD - 1),
                                 tile_position=(0, 32 * j))

        grz = pool.tile([128, 512], F32, name="grz")
        nc.vector.tensor_add(grz[:NP, 0:256], gates[:NP, 0:256],
                             xgt[:NP, 0:256])
        nc.vector.scalar_tensor_tensor(
            grz[:NP, 256:512], gates[:NP, 256:512],
            self.zpad[:NP, ds(iv * U + u + off, 1)], xgt[:NP, 256:512],
            op0=ALU.add, op1=ALU.add)
        rz = pool.tile([128, 512], F32, name="rz")
        nc.scalar.activation(rz[:NP], grz[:NP], AF.Sigmoid)
        t2a = pool.tile([128, 256], F32, name="t2a")
        nc.vector.tensor_add(t2a[:NP], gates[:NP, 512:768], self.bhn[:NP])
        t2 = pool.tile([128, 256], F32, name="t2")
        nc.vector.tensor_mul(t2[:NP], rz[:NP, 0:256], t2a[:NP])
        npre = pool.tile([128, 256], F32, name="npre")
        nc.vector.tensor_add(npre[:NP], t2[:NP], xgt[:NP, 512:768])
        nn = pool.tile([128, 256], F32, name="nn")
        nc.scalar.activation(nn[:NP], npre[:NP], AF.Tanh)
        dlt = pool.tile([128, 256], F32, name="dlt")
        nc.vector.tensor_sub(dlt[:NP], self.hgrp[:NP], nn[:NP])
        e = pool.tile([128, 256], F32, name="e")
        nc.vector.tensor_mul(e[:NP], rz[:NP, 256:512], dlt[:NP])
        nc.vector.tensor_add(self.hgrp[:NP], nn[:NP], e[:NP])

        tp = ppt.tile([128, 256], F32, name="tp")
        for cc in range(2):
            nc.tensor.transpose(tp[:, 128 * cc:128 * cc + NP],
                                self.hgrp[0:NP, 128 * cc:128 * (cc + 1)],
                                ident[0:NP, 0:NP])
        tp4 = tp.rearrange("p (c j r) -> p c j r", c=2, j=G)[:, :, :, 0:B]
        ho = self.hT_hist[:, slot * 64:(slot + 1) * 64]
        ho4 = ho.rearrange("p (c j r) -> p c j r", c=2, j=G)
        nc.scalar.activation(ho4, tp4, AF.Copy)

    def flush(self, iv):
        """Flush h.T for scan steps s = flush_lo + iv*U + [0,U) to
        hT_out t-idx (s-flush_lo) ascending, or n_out-1-(s-flush_lo)
        descending when flush_rev."""
        nc = self.tc.nc
        U = self.U
        hist3 = self.hT_hist.rearrange("p (s x) -> p s x", s=U)
        for k in range(KD):
            base = (k % 2) * 32 + (k // 2) * 8
            src = hist3[:, :, base:base + B]          # [p, slot, b]
            if self.flush_rev:
                dst = self.hT_out[k * 128:(k + 1) * 128,
                                  ds(iv * (-U) + (self.n_out - U), U), :]
                src = src[:, ::-1, :]
            else:
                dst = self.hT_out[k * 128:(k + 1) * 128,
                                  ds(iv * U, U), :]
            nc.sync.dma_start(dst, src)


def build_scan_pair(tc, specs, zeros_st, zrhs, ident, zeros_bf):
    nc = tc.nc
    U = 16
    with contextlib.ExitStack() as c:
        streams = [ScanStream(tc, sp["name"], c, sp["w"], sp["bhn"],
                              sp["zpad"], sp["xg"], sp["hT"],
                              sp["rev_base"], sp["flush_rev"],
                              sp["n_steps"], sp["flush_lo"], zeros_bf, U=U)
                   for sp in specs]
        n_steps = specs[0]["n_steps"]
        flush_lo = specs[0]["flush_lo"]
        assert all(sp["n_steps"] == n_steps and sp["flush_lo"] == flush_lo
                   for sp in specs)
        assert flush_lo % U == 0 and n_steps % U == 0
        nf = flush_lo // U
        if nf > 0:
            with tc.For_i(0, nf) as iv:
                for u in range(U):
                    for s in streams:
                        s.step(iv, u, 0, zeros_st, zrhs, ident)
        with tc.For_i(0, (n_steps - flush_lo) // U) as iv:
            for u in range(U):
                for s in streams:
                    s.step(iv, u, flush_lo, zeros_st, zrhs, ident)
            for s in streams:
                s.flush(iv)


def build_proj(tc, dram, zeros_st, zrhs, ident, nt):
    """x2 = x_own + concat(h1A,h1B) @ gru_out.T; x2nT for FFN."""
    nc = tc.nc
    h1a = dram["hT1_A"].rearrange("d t b -> d (t b)")
    h1b = dram["hT1_B"].rearrange("d t b -> d (t b)")
    own0 = 2 * W * B
    with contextlib.ExitStack() as c:
        wp = c.enter_context(tc.tile_pool(name="pj_w", bufs=1))
        pool = c.enter_context(tc.tile_pool(name="pj_t", bufs=3))
        stp = c.enter_context(tc.tile_pool(name="pj_s", bufs=2))
        pp = c.enter_context(tc.tile_pool(name="pj_p", bufs=4, space="PSUM"))

        gw = wp.tile([128, 2 * KD * D], BF16, name="gw")
        for k in range(2 * KD):
            nc.sync.dma_start(gw[:, k * D:(k + 1) * D],
                              dram["gru_wT"][k * 128:(k + 1) * 128, :])

        with tc.For_i(0, nt) as tv:
            tok = tv * 128
            sts = []
            for k in range(2 * KD):
                stt = stp.tile([128, 128], BF16, name=f"pst{k}")
                srcv = h1a if k < KD else h1b
                kk = k % KD
                nc.sync.dma_start(
                    stt[:], srcv[kk * 128:(kk + 1) * 128, ds(tok, 128)])
                sts.append(stt)
            x2 = pool.tile([128, D], F32, name="x2")
            for cc in range(2):
                ps = pp.tile([128, 512], F32, name="ps")
                nc.tensor.matmul(ps[:], zeros_st[:], zrhs[:],
                                 start=True, stop=False)
                for k in range(2 * KD):
                    nc.tensor.matmul(
                        ps[:], sts[k][:],
                        gw[:, k * D + 512 * cc:k * D + 512 * cc + 512],
                        start=False, stop=(k == 2 * KD - 1))
                xt = pool.tile([128, 512], F32, name="xt")
                nc.sync.dma_start(
                    xt[:], dram["x_win"][ds(tok + own0, 128),
                                         512 * cc:512 * cc + 512])
                nc.vector.tensor_add(x2[:, 512 * cc:512 * cc + 512],
                                     ps[:], xt[:])
            nc.sync.dma_start(dram["x2"][ds(tok, 128), :], x2[:])
            sq = pool.tile([128, D], F32, name="sq")
            ssum = pool.tile([128, 1], F32, name="ssum")
            nc.scalar.activation(sq[:], x2[:], AF.Square, accum_out=ssum[:])
            m = pool.tile([128, 1], F32, name="m")
            nc.vector.tensor_scalar(m[:], ssum[:], 1.0 / D, EPS,
                                    op0=ALU.mult, op1=ALU.add)
            r = pool.tile([128, 1], F32, name="r")
            nc.vector.reciprocal(r[:], m[:])
            s2 = pool.tile([128, 1], F32, name="s2")
            nc.scalar.activation(s2[:], r[:], AF.Sqrt)
            x2n = pool.tile([128, D], F32, name="x2n")
            nc.vector.tensor_scalar_mul(x2n[:], x2[:], s2[:])
            for k in range(KD):
                tpp = pp.tile([128, 128], F32, name="tpp")
                nc.tensor.transpose(tpp[:], x2n[:, k * 128:(k + 1) * 128],
                                    ident[:])
                xc = pool.tile([128, 128], F32R, name="xc")
                nc.scalar.activation(xc[:], tpp[:], AF.Copy)
                nc.sync.dma_start(
                    dram["x2nT"][k * 128:(k + 1) * 128, ds(tok, 128)],
                    xc[:])


def build_ffn13(tc, dram, zeros_st, zrhs, ident, nt):
    nc = tc.nc
    with contextlib.ExitStack() as c:
        wp = c.enter_context(tc.tile_pool(name="fb_w", bufs=1))
        pool = c.enter_context(tc.tile_pool(name="fb_t", bufs=3))
        stp = c.enter_context(tc.tile_pool(name="fb_s", bufs=2))
        pp = c.enter_context(tc.tile_pool(name="fb_p", bufs=2, space="PSUM"))

        w1 = wp.tile([128, KD * FFN], F32R, name="w1")
        w3 = wp.tile([128, KD * FFN], F32R, name="w3")
        for k in range(KD):
            nc.sync.dma_start(w1[:, k * FFN:(k + 1) * FFN],
                              dram["w1T"][k * 128:(k + 1) * 128, :])
            nc.sync.dma_start(w3[:, k * FFN:(k + 1) * FFN],
                              dram["w3T"][k * 128:(k + 1) * 128, :])

        FCH = [(c0, min(512, FFN - c0)) for c0 in range(0, FFN, 512)]
        with tc.For_i(0, nt) as tv:
            tok = tv * 128
            sts = []
            for k in range(KD):
                stt = stp.tile([128, 128], F32R, name=f"bst{k}")
                nc.sync.dma_start(
                    stt[:], dram["x2nT"][k * 128:(k + 1) * 128, ds(tok, 128)])
                sts.append(stt)
            for (c0, cn) in FCH:
                p1 = pp.tile([128, 512], F32, name="p1")
                p3 = pp.tile([128, 512], F32, name="p3")
                nc.tensor.matmul(p1[:, :cn], zeros_st[:], zrhs[:, :cn],
                                 start=True, stop=False)
                nc.tensor.matmul(p3[:, :cn], zeros_st[:], zrhs[:, :cn],
                                 start=True, stop=False)
                for k in range(KD):
                    nc.tensor.matmul(p1[:, :cn], sts[k][:],
                                     w1[:, k * FFN + c0:k * FFN + c0 + cn],
                                     start=False, stop=(k == KD - 1))
                    nc.tensor.matmul(p3[:, :cn], sts[k][:],
                                     w3[:, k * FFN + c0:k * FFN + c0 + cn],
                                     start=False, stop=(k == KD - 1))
                sl = pool.tile([128, 512], F32, name="sl")
                nc.scalar.activation(sl[:, :cn], p1[:, :cn], AF.Silu)
                h1c = pool.tile([128, 512], F32, name="h1c")
                nc.vector.tensor_mul(h1c[:, :cn], sl[:, :cn], p3[:, :cn])
                for q in range(cn // 128):
                    tpp = pp.tile([128, 128], F32, name="tpp")
                    nc.tensor.transpose(
                        tpp[:], h1c[:, q * 128:(q + 1) * 128], ident[:])
                    hc = pool.tile([128, 128], F32R, name="hc")
                    nc.scalar.activation(hc[:], tpp[:], AF.Copy)
                    kf = (c0 + q * 128) // 128
                    nc.sync.dma_start(
                        dram["h1T"][kf * 128:(kf + 1) * 128, ds(tok, 128)],
                        hc[:])


def build_ffn2(tc, dram, zeros_st, zrhs, nt):
    nc = tc.nc
    with contextlib.ExitStack() as c:
        wp = c.enter_context(tc.tile_pool(name="fc_w", bufs=1))
        pool = c.enter_context(tc.tile_pool(name="fc_t", bufs=3))
        stp = c.enter_context(tc.tile_pool(name="fc_s", bufs=2))
        pp = c.enter_context(tc.tile_pool(name="fc_p", bufs=4, space="PSUM"))

        w2 = wp.tile([128, KF * D], F32R, name="w2")
        for k in range(KF):
            nc.sync.dma_start(w2[:, k * D:(k + 1) * D],
                              dram["w2T"][k * 128:(k + 1) * 128, :])

        with tc.For_i(0, nt) as tv:
            tok = tv * 128
            sts = []
            for k in range(KF):
                stt = stp.tile([128, 128], F32R, name=f"cst{k}")
                nc.sync.dma_start(
                    stt[:],
                    dram["h1T"][k * 128:(k + 1) * 128, ds(tok, 128)])
                sts.append(stt)
            for cc in range(2):
                ps = pp.tile([128, 512], F32, name="ps")
                nc.tensor.matmul(ps[:], zeros_st[:], zrhs[:],
                                 start=True, stop=False)
                for k in range(KF):
                    nc.tensor.matmul(
                        ps[:], sts[k][:],
                        w2[:, k * D + 512 * cc:k * D + 512 * cc + 512],
                        start=False, stop=(k == KF - 1))
                xt = pool.tile([128, 512], F32, name="xt")
                nc.sync.dma_start(
                    xt[:], dram["x2"][ds(tok, 128),
                                      512 * cc:512 * cc + 512])
                yo = pool.tile([128, 512], F32, name="yo")
                nc.vector.tensor_add(yo[:], ps[:], xt[:])
                nc.sync.dma_start(
                    dram["y"][ds(tok, 128), 512 * cc:512 * cc + 512],
                    yo[:])


def build_program(nc):
    dram = {}

    def din(name, shape, dt=F32R):
        dram[name] = nc.dram_tensor(name, shape, dt, kind="ExternalInput").ap()

    def dout(name, shape, dt=F32):
        dram[name] = nc.dram_tensor(name, shape, dt,
                                    kind="ExternalOutput").ap()

    def dtmp(name, shape, dt=F32R):
        dram[name] = nc.dram_tensor(name, shape, dt).ap()

    din("x_win", [T0X * B, D], F32)
    din("x_winT", [D, T0X * B])
    for ss in ("A", "B"):
        din(f"wA_{ss}", [D, H3])
        din(f"biasA_{ss}", [128, H3], F32)
        din(f"wD_{ss}", [2 * D, H3], BF16)
        din(f"biasD_{ss}", [128, H3], F32)
        din(f"wS0_{ss}", [128, KD * H3], BF16)
        din(f"bhn0_{ss}", [128, 256], F32)
        din(f"wS1_{ss}", [128, KD * H3], BF16)
        din(f"bhn1_{ss}", [128, 256], F32)
        din(f"zpad0_{ss}", [128, T0], F32)
        din(f"zpad1_{ss}", [128, T1], F32)
    din("zeros", [128, 1024])
    din("zeros_bf", [128, 1024], BF16)
    din("gru_wT", [2 * D, D], BF16)
    din("w1T", [D, FFN])
    din("w3T", [D, FFN])
    din("w2T", [FFN, D])
    dout("y", [L * B, D])

    for ss in ("A", "B"):
        dtmp(f"xg0_{ss}", [T0X * B, G, 768], F32)
        dtmp(f"xg1_{ss}", [T1 * B, G, 768], F32)
        dtmp(f"hT0_{ss}", [D, T0H, B], BF16)
        dtmp(f"hT1_{ss}", [D, L, B], BF16)
    dtmp("x2", [L * B, D], F32)
    dtmp("x2nT", [D, L * B])
    dtmp("h1T", [FFN, L * B])

    NT0 = T0X * B // 128       # 16
    NT1 = T1 * B // 128        # 10
    NTP = L * B // 128         # 8

    with tile.TileContext(nc) as tc:
        with tc.tile_pool(name="consts", bufs=1) as consts:
            zeros_st = consts.tile([1, 128], F32R, name="zeros_st")
            nc.sync.dma_start(zeros_st[:], dram["zeros"][0:1, 0:128])
            zrhs = consts.tile([1, 512], F32R, name="zrhs")
            nc.sync.dma_start(zrhs[:], dram["zeros"][0:1, 0:512])
            ident = consts.tile([128, 128], F32, name="ident")
            make_identity(nc, ident[:])
            s_sb = consts.tile([128, NT0], F32, name="s_sb")

            with nc.named_scope("norm"):
                build_norm_stats(tc, dram["x_win"], s_sb, NT0)
            xt_views = [dram["x_winT"][k * 128:(k + 1) * 128, :]
                        for k in range(KD)]
            with nc.named_scope("xg0"):
                build_xg_gemm(tc, xt_views, KD,
                              [dram["wA_A"], dram["wA_B"]],
                              [dram["biasA_A"], dram["biasA_B"]], s_sb,
                              [dram["xg0_A"], dram["xg0_B"]],
                              zeros_st, zrhs, NT0)
            with nc.named_scope("scan0"):
                build_scan_pair(
                    tc,
                    [dict(name="s0A", w=dram["wS0_A"], bhn=dram["bhn0_A"],
                          zpad=dram["zpad0_A"], xg=dram["xg0_A"],
                          hT=dram["hT0_A"], rev_base=None, flush_rev=False,
                          n_steps=T0, flush_lo=W),
                     dict(name="s0B", w=dram["wS0_B"], bhn=dram["bhn0_B"],
                          zpad=dram["zpad0_B"], xg=dram["xg0_B"],
                          hT=dram["hT0_B"], rev_base=T0X - 1,
                          flush_rev=True, n_steps=T0, flush_lo=W)],
                    zeros_st, zrhs, ident, dram["zeros_bf"])
            # xg1_A covers h0 idx [0,T1) ascending; xg1_B covers idx
            # [W,T0H) ascending (its scan fetches it reversed).
            h0a = dram["hT0_A"].rearrange("d t b -> d (t b)")
            h0b = dram["hT0_B"].rearrange("d t b -> d (t b)")
            for ss, off in (("A", 0), ("B", W * B)):
                sv = [h0a[k * 128:(k + 1) * 128, off:off + T1 * B]
                      for k in range(KD)]
                sv += [h0b[k * 128:(k + 1) * 128, off:off + T1 * B]
                       for k in range(KD)]
                with nc.named_scope(f"xg1{ss}"):
                    build_xg_gemm(tc, sv, 2 * KD, [dram[f"wD_{ss}"]],
                                  [dram[f"biasD_{ss}"]], None,
                                  [dram[f"xg1_{ss}"]],
                                  zeros_st, zrhs, NT1, wdt=BF16, sdt=BF16)
            with nc.named_scope("scan1"):
                build_scan_pair(
                    tc,
                    [dict(name="s1A", w=dram["wS1_A"], bhn=dram["bhn1_A"],
                          zpad=dram["zpad1_A"], xg=dram["xg1_A"],
                          hT=dram["hT1_A"], rev_base=None, flush_rev=False,
                          n_steps=T1, flush_lo=W),
                     dict(name="s1B", w=dram["wS1_B"], bhn=dram["bhn1_B"],
                          zpad=dram["zpad1_B"], xg=dram["xg1_B"],
                          hT=dram["hT1_B"], rev_base=T1 - 1,
                          flush_rev=True, n_steps=T1, flush_lo=W)],
                    zeros_st, zrhs, ident, dram["zeros_bf"])
            with nc.named_scope("proj"):
                build_proj(tc, dram, zeros_st, zrhs, ident, NTP)
            with nc.named_scope("ffn13"):
                build_ffn13(tc, dram, zeros_st, zrhs, ident, NTP)
            with nc.named_scope("ffn2"):
                build_ffn2(tc, dram, zeros_st, zrhs, NTP)
    return dram


# ================================================================== driver
_CACHE = {}


def _prep_shared(inputs):
    import ml_dtypes
    bf = ml_dtypes.bfloat16
    gnw = np.asarray(inputs["gru_norm_w"], np.float32)
    fnw = np.asarray(inputs["ffn_norm_w"], np.float32)
    sh = {}
    for di, ss in ((0, "A"), (1, "B")):
        sh[f"wA_{ss}"] = prep_gemm_weights(
            np.asarray(inputs["w_ih_l0"], np.float32)[di], gnw)
        sh[f"biasA_{ss}"] = prep_gemm_bias(
            np.asarray(inputs["b_ih_l0"], np.float32)[di],
            np.asarray(inputs["b_hh_l0"], np.float32)[di])
        sh[f"wD_{ss}"] = prep_gemm_weights(
            np.asarray(inputs["w_ih_l1"], np.float32)[di]).astype(bf)
        sh[f"biasD_{ss}"] = prep_gemm_bias(
            np.asarray(inputs["b_ih_l1"], np.float32)[di],
            np.asarray(inputs["b_hh_l1"], np.float32)[di])
        for Lx in (0, 1):
            sh[f"wS{Lx}_{ss}"] = prep_scan_weights(
                np.asarray(inputs[f"w_hh_l{Lx}"], np.float32)[di]).astype(bf)
            sh[f"bhn{Lx}_{ss}"] = prep_bhn_scan(
                np.asarray(inputs[f"b_hh_l{Lx}"], np.float32)[di])
    sh["zeros"] = np.zeros((128, 1024), np.float32)
    sh["zeros_bf"] = np.zeros((128, 1024), bf)
    sh["gru_wT"] = np.ascontiguousarray(
        np.asarray(inputs["gru_out_w"], np.float32).T).astype(bf)
    sh["w1T"] = np.ascontiguousarray(
        (np.asarray(inputs["w1"], np.float32) * fnw[None, :]).T)
    sh["w3T"] = np.ascontiguousarray(
        (np.asarray(inputs["w3"], np.float32) * fnw[None, :]).T)
    sh["w2T"] = np.ascontiguousarray(np.asarray(inputs["w2"], np.float32).T)
    return sh


def _host_inputs_spmd(inputs):
    x = np.asarray(inputs["x"], np.float32)          # [B, S, D]
    sh = _prep_shared(inputs)
    im_list = []
    for c in range(NCORE):
        im = dict(sh)
        lo = c * L - 2 * W
        idx = np.arange(lo, lo + T0X)
        valid = (idx >= 0) & (idx < S)
        xw = np.zeros((T0X, B, D), np.float32)
        xw[valid] = x[:, idx[valid], :].transpose(1, 0, 2)
        xw = xw.reshape(T0X * B, D)
        im["x_win"] = np.ascontiguousarray(xw)
        im["x_winT"] = np.ascontiguousarray(xw.T)
        for ss, rev in (("A", False), ("B", True)):
            sidx = np.arange(T0)
            widx = sidx if not rev else (T0X - 1 - sidx)
            t = lo + widx
            z0 = np.zeros((128, T0), np.float32)
            z0[:] = (((t < 0) | (t >= S)) * 40.0)[None, :]
            im[f"zpad0_{ss}"] = z0
            s1 = np.arange(T1)
            hidx = s1 if not rev else (T0H - 1 - s1)
            t1 = (lo + W) + hidx
            z1 = np.zeros((128, T1), np.float32)
            z1[:] = (((t1 < 0) | (t1 >= S)) * 40.0)[None, :]
            im[f"zpad1_{ss}"] = z1
        im_list.append(im)
    return im_list


def get_compiled(n_cores=NCORE):
    if "nc" not in _CACHE:
        nc = bacc.Bacc("TRN2", target_bir_lowering=False, debug=False,
                       num_devices=n_cores)
        build_program(nc)
        nc.compile()
        _CACHE["nc"] = nc
        _CACHE["n_cores"] = n_cores
    return _CACHE["nc"], _CACHE["n_cores"]


def kernel(**inputs) -> np.ndarray:
    im_list = _host_inputs_spmd(inputs)
    nc, n_cores = get_compiled()
    res = run_bass_kernel_spmd(nc, im_list, core_ids=list(range(n_cores)))
    y = np.zeros((B, S, D), np.float32)
    for c in range(n_cores):
        yc = res.results[c]["y"].reshape(L, B, D)
        y[:, c * L:(c + 1) * L, :] = yc.transpose(1, 0, 2)
    return y


# revision 5
# speedup vs baseline: 1.1573x; 1.1573x over previous
"""Trainium2 Bass kernel for nn_BidirectionalGRU (B=8,S=1024,D=1024), v2.

Sharding: 8 cores = 8 time-chunks of L=128 tokens. Every core runs BOTH
GRU directions (streams A=fwd, B=bwd) over its own chunk window with
warm-up margins, so all cross-chunk dependence is absorbed by warm-up
(GRU state contracts; W=32 gives ~1e-7 end-to-end error on CPU) and no
collectives are needed. Per-core differences are input data only
(x windows, z-gate pad masks); the program is SPMD-uniform.

Windows (chunk c, warm-up W, lo = 128c-2W):
  x window:   t in [lo, lo+T0X), T0X=L+4W, stored ascending.
  L0 scan:    A ascends window idx from 0; B descends from T0X-1;
              T0=L+3W steps; h0 flushed for steps>=W into hT0[D,T0H,B]
              stored ascending-t (B's flush reversed), T0H=L+2W,
              covering t in [lo+W, lo+W+T0H).
  xg1:        stream A tokens = h0 idx [0,T1) asc; B = idx [T0H-1..W]
              desc (= its scan order); T1=L+W.
  L1 scan:    both fetch xg1 ascending (already scan-ordered); flush
              steps>=W into hT1[D,L,B] ascending-t (B reversed).
  Out-of-range steps (edge cores) are neutralized by zpad=+40 on the
  z-gate pre-activation (z=1 freezes h=0 exactly); host pads x with 0.
"""
import contextlib
import numpy as np

import concourse.bacc as bacc
import concourse.tile as tile
from concourse import mybir
from concourse.bass import ds
from concourse.bass_utils import run_bass_kernel_spmd
from concourse.masks import make_identity

F32 = mybir.dt.float32
F32R = mybir.dt.float32r
BF16 = mybir.dt.bfloat16
AF = mybir.ActivationFunctionType
ALU = mybir.AluOpType

B, S, D, H3, G, FFN = 8, 1024, 1024, 3072, 4, 2816
NCORE = 8
L = S // NCORE               # 128 owned tokens per core
W = 32                       # warm-up steps
T0X = L + 4 * W              # x window tokens (256)
T0 = L + 3 * W               # L0 scan steps (224)
T0H = L + 2 * W              # h0 stored tokens (192)
T1 = L + W                   # L1 scan steps (160)
KD = D // 128                # 8
KF = FFN // 128              # 22
EPS = 1e-5
NP = 104                     # partitions spanned by grouped layout


# ================================================================ host prep
def gate_perm():
    idx = []
    for j in range(G):
        for blk in range(3):
            base = blk * 1024 + j * 256
            idx.extend(range(base, base + 256))
    return np.array(idx)

PERM = gate_perm()


def prep_scan_weights(w_hh_d):
    wp = w_hh_d[PERM]
    wt = wp.T.reshape(KD, 128, H3).transpose(1, 0, 2)
    return np.ascontiguousarray(wt.reshape(128, KD * H3), dtype=np.float32)


def prep_gemm_weights(w_ih_d, norm_w=None):
    wp = w_ih_d[PERM]
    if norm_w is not None:
        wp = wp * norm_w[None, :]
    return np.ascontiguousarray(wp.T, dtype=np.float32)


def prep_gemm_bias(b_ih_d, b_hh_d):
    bi = b_ih_d[PERM].copy()
    bh = b_hh_d[PERM]
    m = np.where(np.arange(H3) % 768 < 512, bh, 0.0)
    b = (bi + m).astype(np.float32)
    return np.ascontiguousarray(np.broadcast_to(b, (128, H3)), dtype=np.float32)


def prep_bhn_scan(b_hh_d):
    bh = b_hh_d[PERM].reshape(G, 3, 256)[:, 2, :]
    out = np.zeros((128, 256), np.float32)
    for j in range(G):
        out[32 * j:32 * j + 32, :] = bh[j][None, :]
    return out


# ============================================================ device builders
def build_norm_stats(tc, x_nat, s_sb, nt):
    nc = tc.nc
    with tc.tile_pool(name="nstat", bufs=3) as pool:
        for i in range(nt):
            xt = pool.tile([128, D], F32, name="xt")
            nc.sync.dma_start(xt[:], x_nat[i * 128:(i + 1) * 128, :])
            sq = pool.tile([128, D], F32, name="sq")
            ss = pool.tile([128, 1], F32, name="ss")
            nc.scalar.activation(sq[:], xt[:], AF.Square, accum_out=ss[:])
            m = pool.tile([128, 1], F32, name="m")
            nc.vector.tensor_scalar(m[:], ss[:], 1.0 / D, EPS,
                                    op0=ALU.mult, op1=ALU.add)
            r = pool.tile([128, 1], F32, name="r")
            nc.vector.reciprocal(r[:], m[:])
            nc.scalar.activation(s_sb[:, i:i + 1], r[:], AF.Sqrt)


def build_xg_gemm(tc, stat_views, n_k, ws, biases, s_sb, out_vs,
                  zeros_st, zrhs, nt, wdt=BF16, sdt=BF16):
    """out[token, g, 768c] = s*(x @ w) + bias for 1-2 streams sharing
    stationary token tiles. stat_views: n_k APs [128, nt*128].
    Weights are SBUF-resident; stationary loaded once per token tile."""
    nc = tc.nc
    ns = len(ws)
    U = 4
    while nt % U:
        U //= 2
    with contextlib.ExitStack() as c:
        wp = c.enter_context(tc.tile_pool(name="xg_w", bufs=1))
        pool = c.enter_context(tc.tile_pool(name="xg_t", bufs=3))
        stp = c.enter_context(tc.tile_pool(name="xg_s", bufs=2))
        pp = c.enter_context(tc.tile_pool(name="xg_p", bufs=4, space="PSUM"))

        bias_sb = wp.tile([128, ns * H3], F32, name="bias_sb")
        wsb = wp.tile([128, ns * n_k * H3], wdt, name="wsb")
        for si in range(ns):
            nc.sync.dma_start(bias_sb[:, si * H3:(si + 1) * H3],
                              biases[si][:, :])
            for k in range(n_k):
                nc.sync.dma_start(
                    wsb[:, (si * n_k + k) * H3:(si * n_k + k + 1) * H3],
                    ws[si][k * 128:(k + 1) * 128, :])
        with tc.For_i(0, nt // U) as iv:
            for u in range(U):
                tv = iv * U + u
                tok = tv * 128
                sts = []
                for k in range(n_k):
                    stt = stp.tile([128, 128], sdt, name=f"st{k}")
                    nc.sync.dma_start(stt[:],
                                      stat_views[k][:, ds(tok, 128)])
                    sts.append(stt)
                for si in range(ns):
                    for c0 in range(0, H3, 512):
                        ps = pp.tile([128, 512], F32, name="ps")
                        nc.tensor.matmul(ps[:], zeros_st[:], zrhs[:],
                                         start=True, stop=False)
                        for k in range(n_k):
                            wof = (si * n_k + k) * H3 + c0
                            nc.tensor.matmul(
                                ps[:], sts[k][:], wsb[:, wof:wof + 512],
                                start=False, stop=(k == n_k - 1))
                        o = pool.tile([128, 512], F32, name="o")
                        if s_sb is not None:
                            nc.vector.scalar_tensor_tensor(
                                o[:], ps[:], s_sb[:, ds(tv, 1)],
                                bias_sb[:, si * H3 + c0:si * H3 + c0 + 512],
                                op0=ALU.mult, op1=ALU.add)
                        else:
                            nc.vector.tensor_add(
                                o[:], ps[:],
                                bias_sb[:, si * H3 + c0:si * H3 + c0 + 512])
                        cc = c0
                        while cc < c0 + 512:
                            g, gc = divmod(cc, 768)
                            take = min(768 - gc, c0 + 512 - cc)
                            nc.sync.dma_start(
                                out_vs[si][ds(tok, 128), g, gc:gc + take],
                                o[:, cc - c0:cc - c0 + take])
                            cc += take


class ScanStream:
    """State for one of two interleaved GRU scan directions.

    rev_base: None -> xg fetched at storage idx (off + iv*U + u);
              int  -> fetched at (rev_base - (off + iv*U + u)).
    flush_rev: owned h stored descending into hT_out's t axis.
    """

    def __init__(self, tc, name, ctx, w_src, bhn_src, zpad_src, xg_v,
                 hT_out, rev_base, flush_rev, n_steps, flush_lo, zeros_bf,
                 U=16):
        nc = tc.nc
        self.tc = tc
        self.name = name
        self.rev_base = rev_base
        self.flush_rev = flush_rev
        self.n_steps = n_steps
        self.flush_lo = flush_lo
        self.hT_out = hT_out            # [D, n_out*B] flat, ascending t
        self.n_out = hT_out.shape[1] // B
        self.U = U
        wp = ctx.enter_context(tc.tile_pool(name=f"w_{name}", bufs=1))
        st = ctx.enter_context(tc.tile_pool(name=f"s_{name}", bufs=1))
        self.pool = ctx.enter_context(tc.tile_pool(name=f"t_{name}", bufs=3))
        self.pp = ctx.enter_context(
            tc.tile_pool(name=f"p_{name}", bufs=1, space="PSUM"))
        self.ppt = ctx.enter_context(
            tc.tile_pool(name=f"pt_{name}", bufs=1, space="PSUM"))

        self.w_sb = wp.tile([128, KD * H3], BF16, name="w_sb")
        nc.sync.dma_start(self.w_sb[:], w_src[:, :])
        self.bhn = wp.tile([128, 256], F32, name="bhn")
        nc.sync.dma_start(self.bhn[:], bhn_src[:, :])
        self.zpad = wp.tile([128, n_steps], F32, name="zpad")
        nc.sync.dma_start(self.zpad[:], zpad_src[:, 0:n_steps])

        self.hgrp = st.tile([128, 256], F32, name="hgrp")
        nc.gpsimd.memset(self.hgrp[:], 0.0)
        self.hT_hist = st.tile([128, U * 64], BF16, name="hT_hist")
        nc.sync.dma_start(self.hT_hist[:], zeros_bf[:, 0:U * 64])
        self.xg_t = xg_v.rearrange("(t b) g c -> t g b c", b=B)

    def step(self, iv, u, off, zeros_st, zrhs, ident):
        nc = self.tc.nc
        pool, pp, ppt = self.pool, self.pp, self.ppt
        U = self.U
        slot, pslot = u, (u - 1) % U
        if self.rev_base is None:
            t_el = iv * U + (u + off)
        else:
            t_el = iv * (-U) + (self.rev_base - u - off)
        xgt = pool.tile([128, 768], F32, name="xgt")
        for j in range(G):
            srcj = self.xg_t[ds(t_el, 1), j, :, :].rearrange(
                "a b c -> (a b) c")
            nc.sync.dma_start(xgt[32 * j:32 * j + B, :], srcj)

        gates = pp.tile([128, 768], F32, name="gates")
        nc.tensor.matmul(gates[:, 0:512], zeros_st[:], zrhs[:],
                         start=True, stop=False)
        nc.tensor.matmul(gates[:, 512:768], zeros_st[:], zrhs[:, 0:256],
                         start=True, stop=False)
        for k in range(KD):
            j2, c2 = divmod(k, 2)
            lof = pslot * 64 + c2 * 32 + j2 * 8
            lhsT = self.hT_hist[:, lof:lof + 8]
            for j in range(G):
                wof = k * H3 + j * 768
                nc.tensor.matmul(gates[32 * j:32 * j + 8, 0:512], lhsT,
                                 self.w_sb[:, wof:wof + 512],
                                 start=False, stop=False,
                                 tile_position=(0, 32 * j))
                nc.tensor.matmul(gates[32 * j:32 * j + 8, 512:768], lhsT,
                                 self.w_sb[:, wof + 512:wof + 768],
                                 start=False, stop=(k == KD - 1),
                                 tile_position=(0, 32 * j))

        grz = pool.tile([128, 512], F32, name="grz")
        nc.vector.tensor_add(grz[:NP, 0:256], gates[:NP, 0:256],
                             xgt[:NP, 0:256])
        nc.vector.scalar_tensor_tensor(
            grz[:NP, 256:512], gates[:NP, 256:512],
            self.zpad[:NP, ds(iv * U + u + off, 1)], xgt[:NP, 256:512],
            op0=ALU.add, op1=ALU.add)
        rz = pool.tile([128, 512], F32, name="rz")
        nc.scalar.activation(rz[:NP], grz[:NP], AF.Sigmoid)
        t2a = pool.tile([128, 256], F32, name="t2a")
        nc.vector.tensor_add(t2a[:NP], gates[:NP, 512:768], self.bhn[:NP])
        t2 = pool.tile([128, 256], F32, name="t2")
        nc.vector.tensor_mul(t2[:NP], rz[:NP, 0:256], t2a[:NP])
        npre = pool.tile([128, 256], F32, name="npre")
        nc.vector.tensor_add(npre[:NP], t2[:NP], xgt[:NP, 512:768])
        nn = pool.tile([128, 256], F32, name="nn")
        nc.scalar.activation(nn[:NP], npre[:NP], AF.Tanh)
        dlt = pool.tile([128, 256], F32, name="dlt")
        nc.vector.tensor_sub(dlt[:NP], self.hgrp[:NP], nn[:NP])
        e = pool.tile([128, 256], F32, name="e")
        nc.vector.tensor_mul(e[:NP], rz[:NP, 256:512], dlt[:NP])
        nc.vector.tensor_add(self.hgrp[:NP], nn[:NP], e[:NP])

        tp = ppt.tile([128, 256], F32, name="tp")
        for cc in range(2):
            nc.tensor.transpose(tp[:, 128 * cc:128 * cc + NP],
                                self.hgrp[0:NP, 128 * cc:128 * (cc + 1)],
                                ident[0:NP, 0:NP])
        tp4 = tp.rearrange("p (c j r) -> p c j r", c=2, j=G)[:, :, :, 0:B]
        ho = self.hT_hist[:, slot * 64:(slot + 1) * 64]
        ho4 = ho.rearrange("p (c j r) -> p c j r", c=2, j=G)
        nc.scalar.activation(ho4, tp4, AF.Copy)

    def flush(self, iv):
        """Flush h.T for scan steps s = flush_lo + iv*U + [0,U) to
        hT_out t-idx (s-flush_lo) ascending, or n_out-1-(s-flush_lo)
        descending when flush_rev."""
        nc = self.tc.nc
        U = self.U
        UB = U * B
        hist3 = self.hT_hist.rearrange("p (s x) -> p s x", s=U)
        for k in range(KD):
            base = (k % 2) * 32 + (k // 2) * 8
            src = hist3[:, :, base:base + B]          # [p, slot, b]
            if self.flush_rev:
                dst = self.hT_out[k * 128:(k + 1) * 128,
                                  ds(iv * (-UB) + (self.n_out - U) * B, UB)]
                src = src[:, ::-1, :]
            else:
                dst = self.hT_out[k * 128:(k + 1) * 128, ds(iv * UB, UB)]
            nc.sync.dma_start(dst, src)


def build_scan_pair(tc, specs, zeros_st, zrhs, ident, zeros_bf):
    nc = tc.nc
    U = 16
    with contextlib.ExitStack() as c:
        streams = [ScanStream(tc, sp["name"], c, sp["w"], sp["bhn"],
                              sp["zpad"], sp["xg"], sp["hT"],
                              sp["rev_base"], sp["flush_rev"],
                              sp["n_steps"], sp["flush_lo"], zeros_bf, U=U)
                   for sp in specs]
        n_steps = specs[0]["n_steps"]
        flush_lo = specs[0]["flush_lo"]
        assert all(sp["n_steps"] == n_steps and sp["flush_lo"] == flush_lo
                   for sp in specs)
        assert flush_lo % U == 0 and n_steps % U == 0
        nf = flush_lo // U
        if nf > 0:
            with tc.For_i(0, nf) as iv:
                for u in range(U):
                    for s in streams:
                        s.step(iv, u, 0, zeros_st, zrhs, ident)
        with tc.For_i(0, (n_steps - flush_lo) // U) as iv:
            for u in range(U):
                for s in streams:
                    s.step(iv, u, flush_lo, zeros_st, zrhs, ident)
            for s in streams:
                s.flush(iv)


def build_proj(tc, dram, zeros_st, zrhs, ident, nt):
    """x2 = x_own + concat(h1A,h1B) @ gru_out.T; x2nT for FFN."""
    nc = tc.nc
    h1a = dram["hT1_A"]
    h1b = dram["hT1_B"]
    own0 = 2 * W * B
    with contextlib.ExitStack() as c:
        wp = c.enter_context(tc.tile_pool(name="pj_w", bufs=1))
        pool = c.enter_context(tc.tile_pool(name="pj_t", bufs=3))
        stp = c.enter_context(tc.tile_pool(name="pj_s", bufs=2))
        pp = c.enter_context(tc.tile_pool(name="pj_p", bufs=4, space="PSUM"))

        gw = wp.tile([128, 2 * KD * D], BF16, name="gw")
        for k in range(2 * KD):
            nc.sync.dma_start(gw[:, k * D:(k + 1) * D],
                              dram["gru_wT"][k * 128:(k + 1) * 128, :])

        with tc.For_i(0, nt) as tv:
            tok = tv * 128
            sts = []
            for k in range(2 * KD):
                stt = stp.tile([128, 128], BF16, name=f"pst{k}")
                srcv = h1a if k < KD else h1b
                kk = k % KD
                nc.sync.dma_start(
                    stt[:], srcv[kk * 128:(kk + 1) * 128, ds(tok, 128)])
                sts.append(stt)
            x2 = pool.tile([128, D], F32, name="x2")
            for cc in range(2):
                ps = pp.tile([128, 512], F32, name="ps")
                nc.tensor.matmul(ps[:], zeros_st[:], zrhs[:],
                                 start=True, stop=False)
                for k in range(2 * KD):
                    nc.tensor.matmul(
                        ps[:], sts[k][:],
                        gw[:, k * D + 512 * cc:k * D + 512 * cc + 512],
                        start=False, stop=(k == 2 * KD - 1))
                xt = pool.tile([128, 512], F32, name="xt")
                nc.sync.dma_start(
                    xt[:], dram["x_win"][ds(tok + own0, 128),
                                         512 * cc:512 * cc + 512])
                nc.vector.tensor_add(x2[:, 512 * cc:512 * cc + 512],
                                     ps[:], xt[:])
            nc.sync.dma_start(dram["x2"][ds(tok, 128), :], x2[:])
            sq = pool.tile([128, D], F32, name="sq")
            ssum = pool.tile([128, 1], F32, name="ssum")
            nc.scalar.activation(sq[:], x2[:], AF.Square, accum_out=ssum[:])
            m = pool.tile([128, 1], F32, name="m")
            nc.vector.tensor_scalar(m[:], ssum[:], 1.0 / D, EPS,
                                    op0=ALU.mult, op1=ALU.add)
            r = pool.tile([128, 1], F32, name="r")
            nc.vector.reciprocal(r[:], m[:])
            s2 = pool.tile([128, 1], F32, name="s2")
            nc.scalar.activation(s2[:], r[:], AF.Sqrt)
            x2n = pool.tile([128, D], F32, name="x2n")
            nc.vector.tensor_scalar_mul(x2n[:], x2[:], s2[:])
            for k in range(KD):
                tpp = pp.tile([128, 128], F32, name="tpp")
                nc.tensor.transpose(tpp[:], x2n[:, k * 128:(k + 1) * 128],
                                    ident[:])
                xc = pool.tile([128, 128], F32R, name="xc")
                nc.scalar.activation(xc[:], tpp[:], AF.Copy)
                nc.sync.dma_start(
                    dram["x2nT"][k * 128:(k + 1) * 128, ds(tok, 128)],
                    xc[:])


def build_ffn13(tc, dram, zeros_st, zrhs, ident, nt):
    nc = tc.nc
    with contextlib.ExitStack() as c:
        wp = c.enter_context(tc.tile_pool(name="fb_w", bufs=1))
        pool = c.enter_context(tc.tile_pool(name="fb_t", bufs=3))
        stp = c.enter_context(tc.tile_pool(name="fb_s", bufs=2))
        pp = c.enter_context(tc.tile_pool(name="fb_p", bufs=2, space="PSUM"))

        w1 = wp.tile([128, KD * FFN], F32R, name="w1")
        w3 = wp.tile([128, KD * FFN], F32R, name="w3")
        for k in range(KD):
            nc.sync.dma_start(w1[:, k * FFN:(k + 1) * FFN],
                              dram["w1T"][k * 128:(k + 1) * 128, :])
            nc.sync.dma_start(w3[:, k * FFN:(k + 1) * FFN],
                              dram["w3T"][k * 128:(k + 1) * 128, :])

        FCH = [(c0, min(512, FFN - c0)) for c0 in range(0, FFN, 512)]
        with tc.For_i(0, nt) as tv:
            tok = tv * 128
            sts = []
            for k in range(KD):
                stt = stp.tile([128, 128], F32R, name=f"bst{k}")
                nc.sync.dma_start(
                    stt[:], dram["x2nT"][k * 128:(k + 1) * 128, ds(tok, 128)])
                sts.append(stt)
            for (c0, cn) in FCH:
                p1 = pp.tile([128, 512], F32, name="p1")
                p3 = pp.tile([128, 512], F32, name="p3")
                nc.tensor.matmul(p1[:, :cn], zeros_st[:], zrhs[:, :cn],
                                 start=True, stop=False)
                nc.tensor.matmul(p3[:, :cn], zeros_st[:], zrhs[:, :cn],
                                 start=True, stop=False)
                for k in range(KD):
                    nc.tensor.matmul(p1[:, :cn], sts[k][:],
                                     w1[:, k * FFN + c0:k * FFN + c0 + cn],
                                     start=False, stop=(k == KD - 1))
                    nc.tensor.matmul(p3[:, :cn], sts[k][:],
                                     w3[:, k * FFN + c0:k * FFN + c0 + cn],
                                     start=False, stop=(k == KD - 1))
                sl = pool.tile([128, 512], F32, name="sl")
                nc.scalar.activation(sl[:, :cn], p1[:, :cn], AF.Silu)
                h1c = pool.tile([128, 512], F32, name="h1c")
                nc.vector.tensor_mul(h1c[:, :cn], sl[:, :cn], p3[:, :cn])
                for q in range(cn // 128):
                    tpp = pp.tile([128, 128], F32, name="tpp")
                    nc.tensor.transpose(
                        tpp[:], h1c[:, q * 128:(q + 1) * 128], ident[:])
                    hc = pool.tile([128, 128], F32R, name="hc")
                    nc.scalar.activation(hc[:], tpp[:], AF.Copy)
                    kf = (c0 + q * 128) // 128
                    nc.sync.dma_start(
                        dram["h1T"][kf * 128:(kf + 1) * 128, ds(tok, 128)],
                        hc[:])


def build_ffn2(tc, dram, zeros_st, zrhs, nt):
    nc = tc.nc
    with contextlib.ExitStack() as c:
        wp = c.enter_context(tc.tile_pool(name="fc_w", bufs=1))
        pool = c.enter_context(tc.tile_pool(name="fc_t", bufs=3))
        stp = c.enter_context(tc.tile_pool(name="fc_s", bufs=2))
        pp = c.enter_context(tc.tile_pool(name="fc_p", bufs=4, space="PSUM"))

        w2 = wp.tile([128, KF * D], F32R, name="w2")
        for k in range(KF):
            nc.sync.dma_start(w2[:, k * D:(k + 1) * D],
                              dram["w2T"][k * 128:(k + 1) * 128, :])

        with tc.For_i(0, nt) as tv:
            tok = tv * 128
            sts = []
            for k in range(KF):
                stt = stp.tile([128, 128], F32R, name=f"cst{k}")
                nc.sync.dma_start(
                    stt[:],
                    dram["h1T"][k * 128:(k + 1) * 128, ds(tok, 128)])
                sts.append(stt)
            for cc in range(2):
                ps = pp.tile([128, 512], F32, name="ps")
                nc.tensor.matmul(ps[:], zeros_st[:], zrhs[:],
                                 start=True, stop=False)
                for k in range(KF):
                    nc.tensor.matmul(
                        ps[:], sts[k][:],
                        w2[:, k * D + 512 * cc:k * D + 512 * cc + 512],
                        start=False, stop=(k == KF - 1))
                xt = pool.tile([128, 512], F32, name="xt")
                nc.sync.dma_start(
                    xt[:], dram["x2"][ds(tok, 128),
                                      512 * cc:512 * cc + 512])
                yo = pool.tile([128, 512], F32, name="yo")
                nc.vector.tensor_add(yo[:], ps[:], xt[:])
                nc.sync.dma_start(
                    dram["y"][ds(tok, 128), 512 * cc:512 * cc + 512],
                    yo[:])


def build_program(nc):
    dram = {}

    def din(name, shape, dt=F32R):
        dram[name] = nc.dram_tensor(name, shape, dt, kind="ExternalInput").ap()

    def dout(name, shape, dt=F32):
        dram[name] = nc.dram_tensor(name, shape, dt,
                                    kind="ExternalOutput").ap()

    def dtmp(name, shape, dt=F32R):
        dram[name] = nc.dram_tensor(name, shape, dt).ap()

    din("x_win", [T0X * B, D], F32)
    din("x_winT", [D, T0X * B], BF16)
    for ss in ("A", "B"):
        din(f"wA_{ss}", [D, H3], BF16)
        din(f"biasA_{ss}", [128, H3], F32)
        din(f"wD_{ss}", [2 * D, H3], BF16)
        din(f"biasD_{ss}", [128, H3], F32)
        din(f"wS0_{ss}", [128, KD * H3], BF16)
        din(f"bhn0_{ss}", [128, 256], F32)
        din(f"wS1_{ss}", [128, KD * H3], BF16)
        din(f"bhn1_{ss}", [128, 256], F32)
        din(f"zpad0_{ss}", [128, T0], F32)
        din(f"zpad1_{ss}", [128, T1], F32)
    din("zeros", [128, 1024])
    din("zeros_bf", [128, 1024], BF16)
    din("gru_wT", [2 * D, D], BF16)
    din("w1T", [D, FFN])
    din("w3T", [D, FFN])
    din("w2T", [FFN, D])
    dout("y", [L * B, D])

    for ss in ("A", "B"):
        dtmp(f"xg0_{ss}", [T0X * B, G, 768], F32)
        dtmp(f"xg1_{ss}", [T1 * B, G, 768], F32)
        dtmp(f"hT0_{ss}", [D, T0H * B], BF16)
        dtmp(f"hT1_{ss}", [D, L * B], BF16)
    dtmp("x2", [L * B, D], F32)
    dtmp("x2nT", [D, L * B])
    dtmp("h1T", [FFN, L * B])

    NT0 = T0X * B // 128       # 16
    NT1 = T1 * B // 128        # 10
    NTP = L * B // 128         # 8

    with tile.TileContext(nc) as tc:
        with tc.tile_pool(name="consts", bufs=1) as consts:
            zeros_st = consts.tile([1, 128], F32R, name="zeros_st")
            nc.sync.dma_start(zeros_st[:], dram["zeros"][0:1, 0:128])
            zrhs = consts.tile([1, 512], F32R, name="zrhs")
            nc.sync.dma_start(zrhs[:], dram["zeros"][0:1, 0:512])
            ident = consts.tile([128, 128], F32, name="ident")
            make_identity(nc, ident[:])
            s_sb = consts.tile([128, NT0], F32, name="s_sb")

            with nc.named_scope("norm"):
                build_norm_stats(tc, dram["x_win"], s_sb, NT0)
            xt_views = [dram["x_winT"][k * 128:(k + 1) * 128, :]
                        for k in range(KD)]
            with nc.named_scope("xg0"):
                build_xg_gemm(tc, xt_views, KD,
                              [dram["wA_A"], dram["wA_B"]],
                              [dram["biasA_A"], dram["biasA_B"]], s_sb,
                              [dram["xg0_A"], dram["xg0_B"]],
                              zeros_st, zrhs, NT0)
            with nc.named_scope("scan0"):
                build_scan_pair(
                    tc,
                    [dict(name="s0A", w=dram["wS0_A"], bhn=dram["bhn0_A"],
                          zpad=dram["zpad0_A"], xg=dram["xg0_A"],
                          hT=dram["hT0_A"], rev_base=None, flush_rev=False,
                          n_steps=T0, flush_lo=W),
                     dict(name="s0B", w=dram["wS0_B"], bhn=dram["bhn0_B"],
                          zpad=dram["zpad0_B"], xg=dram["xg0_B"],
                          hT=dram["hT0_B"], rev_base=T0X - 1,
                          flush_rev=True, n_steps=T0, flush_lo=W)],
                    zeros_st, zrhs, ident, dram["zeros_bf"])
            # xg1_A covers h0 idx [0,T1) ascending; xg1_B covers idx
            # [W,T0H) ascending (its scan fetches it reversed).
            h0a = dram["hT0_A"]
            h0b = dram["hT0_B"]
            for ss, off in (("A", 0), ("B", W * B)):
                sv = [h0a[k * 128:(k + 1) * 128, off:off + T1 * B]
                      for k in range(KD)]
                sv += [h0b[k * 128:(k + 1) * 128, off:off + T1 * B]
                       for k in range(KD)]
                with nc.named_scope(f"xg1{ss}"):
                    build_xg_gemm(tc, sv, 2 * KD, [dram[f"wD_{ss}"]],
                                  [dram[f"biasD_{ss}"]], None,
                                  [dram[f"xg1_{ss}"]],
                                  zeros_st, zrhs, NT1, wdt=BF16, sdt=BF16)
            with nc.named_scope("scan1"):
                build_scan_pair(
                    tc,
                    [dict(name="s1A", w=dram["wS1_A"], bhn=dram["bhn1_A"],
                          zpad=dram["zpad1_A"], xg=dram["xg1_A"],
                          hT=dram["hT1_A"], rev_base=None, flush_rev=False,
                          n_steps=T1, flush_lo=W),
                     dict(name="s1B", w=dram["wS1_B"], bhn=dram["bhn1_B"],
                          zpad=dram["zpad1_B"], xg=dram["xg1_B"],
                          hT=dram["hT1_B"], rev_base=T1 - 1,
                          flush_rev=True, n_steps=T1, flush_lo=W)],
                    zeros_st, zrhs, ident, dram["zeros_bf"])
            with nc.named_scope("proj"):
                build_proj(tc, dram, zeros_st, zrhs, ident, NTP)
            with nc.named_scope("ffn13"):
                build_ffn13(tc, dram, zeros_st, zrhs, ident, NTP)
            with nc.named_scope("ffn2"):
                build_ffn2(tc, dram, zeros_st, zrhs, NTP)
    return dram


# ================================================================== driver
_CACHE = {}


def _prep_shared(inputs):
    import ml_dtypes
    bf = ml_dtypes.bfloat16
    gnw = np.asarray(inputs["gru_norm_w"], np.float32)
    fnw = np.asarray(inputs["ffn_norm_w"], np.float32)
    sh = {}
    for di, ss in ((0, "A"), (1, "B")):
        sh[f"wA_{ss}"] = prep_gemm_weights(
            np.asarray(inputs["w_ih_l0"], np.float32)[di], gnw).astype(bf)
        sh[f"biasA_{ss}"] = prep_gemm_bias(
            np.asarray(inputs["b_ih_l0"], np.float32)[di],
            np.asarray(inputs["b_hh_l0"], np.float32)[di])
        sh[f"wD_{ss}"] = prep_gemm_weights(
            np.asarray(inputs["w_ih_l1"], np.float32)[di]).astype(bf)
        sh[f"biasD_{ss}"] = prep_gemm_bias(
            np.asarray(inputs["b_ih_l1"], np.float32)[di],
            np.asarray(inputs["b_hh_l1"], np.float32)[di])
        for Lx in (0, 1):
            sh[f"wS{Lx}_{ss}"] = prep_scan_weights(
                np.asarray(inputs[f"w_hh_l{Lx}"], np.float32)[di]).astype(bf)
            sh[f"bhn{Lx}_{ss}"] = prep_bhn_scan(
                np.asarray(inputs[f"b_hh_l{Lx}"], np.float32)[di])
    sh["zeros"] = np.zeros((128, 1024), np.float32)
    sh["zeros_bf"] = np.zeros((128, 1024), bf)
    sh["gru_wT"] = np.ascontiguousarray(
        np.asarray(inputs["gru_out_w"], np.float32).T).astype(bf)
    sh["w1T"] = np.ascontiguousarray(
        (np.asarray(inputs["w1"], np.float32) * fnw[None, :]).T)
    sh["w3T"] = np.ascontiguousarray(
        (np.asarray(inputs["w3"], np.float32) * fnw[None, :]).T)
    sh["w2T"] = np.ascontiguousarray(np.asarray(inputs["w2"], np.float32).T)
    return sh


def _host_inputs_spmd(inputs):
    x = np.asarray(inputs["x"], np.float32)          # [B, S, D]
    sh = _prep_shared(inputs)
    im_list = []
    for c in range(NCORE):
        im = dict(sh)
        lo = c * L - 2 * W
        idx = np.arange(lo, lo + T0X)
        valid = (idx >= 0) & (idx < S)
        xw = np.zeros((T0X, B, D), np.float32)
        xw[valid] = x[:, idx[valid], :].transpose(1, 0, 2)
        xw = xw.reshape(T0X * B, D)
        im["x_win"] = np.ascontiguousarray(xw)
        import ml_dtypes
        im["x_winT"] = np.ascontiguousarray(xw.T).astype(ml_dtypes.bfloat16)
        for ss, rev in (("A", False), ("B", True)):
            sidx = np.arange(T0)
            widx = sidx if not rev else (T0X - 1 - sidx)
            t = lo + widx
            z0 = np.zeros((128, T0), np.float32)
            z0[:] = (((t < 0) | (t >= S)) * 40.0)[None, :]
            im[f"zpad0_{ss}"] = z0
            s1 = np.arange(T1)
            hidx = s1 if not rev else (T0H - 1 - s1)
            t1 = (lo + W) + hidx
            z1 = np.zeros((128, T1), np.float32)
            z1[:] = (((t1 < 0) | (t1 >= S)) * 40.0)[None, :]
            im[f"zpad1_{ss}"] = z1
        im_list.append(im)
    return im_list


def get_compiled(n_cores=NCORE):
    if "nc" not in _CACHE:
        nc = bacc.Bacc("TRN2", target_bir_lowering=False, debug=False,
                       num_devices=n_cores)
        build_program(nc)
        nc.compile()
        _CACHE["nc"] = nc
        _CACHE["n_cores"] = n_cores
    return _CACHE["nc"], _CACHE["n_cores"]


def kernel(**inputs) -> np.ndarray:
    im_list = _host_inputs_spmd(inputs)
    nc, n_cores = get_compiled()
    res = run_bass_kernel_spmd(nc, im_list, core_ids=list(range(n_cores)))
    y = np.zeros((B, S, D), np.float32)
    for c in range(n_cores):
        yc = res.results[c]["y"].reshape(L, B, D)
        y[:, c * L:(c + 1) * L, :] = yc.transpose(1, 0, 2)
    return y


# revision 6
# speedup vs baseline: 1.8488x; 1.5975x over previous
"""Trainium2 Bass kernel for nn_BidirectionalGRU (B=8,S=1024,D=1024), v2.

Sharding: 8 cores = 8 time-chunks of L=128 tokens. Every core runs BOTH
GRU directions (streams A=fwd, B=bwd) over its own chunk window with
warm-up margins, so all cross-chunk dependence is absorbed by warm-up
(GRU state contracts; W=32 gives ~1e-7 end-to-end error on CPU) and no
collectives are needed. Per-core differences are input data only
(x windows, z-gate pad masks); the program is SPMD-uniform.

Windows (chunk c, warm-up W, lo = 128c-2W):
  x window:   t in [lo, lo+T0X), T0X=L+4W, stored ascending.
  L0 scan:    A ascends window idx from 0; B descends from T0X-1;
              T0=L+3W steps; h0 flushed for steps>=W into hT0[D,T0H,B]
              stored ascending-t (B's flush reversed), T0H=L+2W,
              covering t in [lo+W, lo+W+T0H).
  xg1:        stream A tokens = h0 idx [0,T1) asc; B = idx [T0H-1..W]
              desc (= its scan order); T1=L+W.
  L1 scan:    both fetch xg1 ascending (already scan-ordered); flush
              steps>=W into hT1[D,L,B] ascending-t (B reversed).
  Out-of-range steps (edge cores) are neutralized by zpad=+40 on the
  z-gate pre-activation (z=1 freezes h=0 exactly); host pads x with 0.
"""
import contextlib
import numpy as np

import concourse.bacc as bacc
import concourse.tile as tile
from concourse import mybir
from concourse.bass import ds
from concourse.bass_utils import run_bass_kernel_spmd
from concourse.masks import make_identity

F32 = mybir.dt.float32
F32R = mybir.dt.float32r
BF16 = mybir.dt.bfloat16
AF = mybir.ActivationFunctionType
ALU = mybir.AluOpType

B, S, D, H3, G, FFN = 8, 1024, 1024, 3072, 4, 2816
NCORE = 8
L = S // NCORE               # 128 owned tokens per core
W = 16                       # warm-up steps
T0X = L + 4 * W              # x window tokens (256)
T0 = L + 3 * W               # L0 scan steps (224)
T0H = L + 2 * W              # h0 stored tokens (192)
T1 = L + W                   # L1 scan steps (160)
KD = D // 128                # 8
KF = FFN // 128              # 22
EPS = 1e-5
NP = 104                     # partitions spanned by grouped layout


# ================================================================ host prep
def gate_perm():
    idx = []
    for j in range(G):
        for blk in range(3):
            base = blk * 1024 + j * 256
            idx.extend(range(base, base + 256))
    return np.array(idx)

PERM = gate_perm()


def prep_scan_weights(w_hh_d):
    wp = w_hh_d[PERM]
    wt = wp.T.reshape(KD, 128, H3).transpose(1, 0, 2)
    return np.ascontiguousarray(wt.reshape(128, KD * H3), dtype=np.float32)


def prep_gemm_weights(w_ih_d, norm_w=None):
    wp = w_ih_d[PERM]
    if norm_w is not None:
        wp = wp * norm_w[None, :]
    return np.ascontiguousarray(wp.T, dtype=np.float32)


def prep_gemm_bias(b_ih_d, b_hh_d):
    bi = b_ih_d[PERM].copy()
    bh = b_hh_d[PERM]
    m = np.where(np.arange(H3) % 768 < 512, bh, 0.0)
    b = (bi + m).astype(np.float32)
    return np.ascontiguousarray(np.broadcast_to(b, (128, H3)), dtype=np.float32)


def prep_bhn_scan(b_hh_d):
    bh = b_hh_d[PERM].reshape(G, 3, 256)[:, 2, :]
    out = np.zeros((128, 256), np.float32)
    for j in range(G):
        out[32 * j:32 * j + 32, :] = bh[j][None, :]
    return out


# ============================================================ device builders
def build_norm_stats(tc, x_nat, s_sb, nt):
    nc = tc.nc
    with tc.tile_pool(name="nstat", bufs=3) as pool:
        for i in range(nt):
            xt = pool.tile([128, D], F32, name="xt")
            nc.sync.dma_start(xt[:], x_nat[i * 128:(i + 1) * 128, :])
            sq = pool.tile([128, D], F32, name="sq")
            ss = pool.tile([128, 1], F32, name="ss")
            nc.scalar.activation(sq[:], xt[:], AF.Square, accum_out=ss[:])
            m = pool.tile([128, 1], F32, name="m")
            nc.vector.tensor_scalar(m[:], ss[:], 1.0 / D, EPS,
                                    op0=ALU.mult, op1=ALU.add)
            r = pool.tile([128, 1], F32, name="r")
            nc.vector.reciprocal(r[:], m[:])
            nc.scalar.activation(s_sb[:, i:i + 1], r[:], AF.Sqrt)


def build_xg_gemm(tc, stat_views, n_k, ws, biases, s_sb, out_vs,
                  zeros_st, zrhs, nt, wdt=BF16, sdt=BF16):
    """out[token, g, 768c] = s*(x @ w) + bias for 1-2 streams sharing
    stationary token tiles. stat_views: n_k APs [128, nt*128].
    Weights are SBUF-resident; stationary loaded once per token tile."""
    nc = tc.nc
    ns = len(ws)
    U = 4
    while nt % U:
        U //= 2
    with contextlib.ExitStack() as c:
        wp = c.enter_context(tc.tile_pool(name="xg_w", bufs=1))
        pool = c.enter_context(tc.tile_pool(name="xg_t", bufs=3))
        stp = c.enter_context(tc.tile_pool(name="xg_s", bufs=2))
        pp = c.enter_context(tc.tile_pool(name="xg_p", bufs=4, space="PSUM"))

        bias_sb = wp.tile([128, ns * H3], F32, name="bias_sb")
        wsb = wp.tile([128, ns * n_k * H3], wdt, name="wsb")
        for si in range(ns):
            nc.sync.dma_start(bias_sb[:, si * H3:(si + 1) * H3],
                              biases[si][:, :])
            for k in range(n_k):
                nc.sync.dma_start(
                    wsb[:, (si * n_k + k) * H3:(si * n_k + k + 1) * H3],
                    ws[si][k * 128:(k + 1) * 128, :])
        with tc.For_i(0, nt // U) as iv:
            for u in range(U):
                tv = iv * U + u
                tok = tv * 128
                sts = []
                for k in range(n_k):
                    stt = stp.tile([128, 128], sdt, name=f"st{k}")
                    nc.sync.dma_start(stt[:],
                                      stat_views[k][:, ds(tok, 128)])
                    sts.append(stt)
                for si in range(ns):
                    for c0 in range(0, H3, 512):
                        ps = pp.tile([128, 512], F32, name="ps")
                        nc.tensor.matmul(ps[:], zeros_st[:], zrhs[:],
                                         start=True, stop=False)
                        for k in range(n_k):
                            wof = (si * n_k + k) * H3 + c0
                            nc.tensor.matmul(
                                ps[:], sts[k][:], wsb[:, wof:wof + 512],
                                start=False, stop=(k == n_k - 1))
                        o = pool.tile([128, 512], BF16, name="o")
                        if s_sb is not None:
                            nc.vector.scalar_tensor_tensor(
                                o[:], ps[:], s_sb[:, ds(tv, 1)],
                                bias_sb[:, si * H3 + c0:si * H3 + c0 + 512],
                                op0=ALU.mult, op1=ALU.add)
                        else:
                            nc.vector.tensor_add(
                                o[:], ps[:],
                                bias_sb[:, si * H3 + c0:si * H3 + c0 + 512])
                        cc = c0
                        while cc < c0 + 512:
                            g, gc = divmod(cc, 768)
                            take = min(768 - gc, c0 + 512 - cc)
                            dstv = out_vs[si][ds(tv, 1), g, :, :,
                                              gc:gc + take]
                            dstv = dstv.rearrange("a b u c -> a u b c")
                            nc.sync.dma_start(
                                dstv, o[:, cc - c0:cc - c0 + take])
                            cc += take


class ScanStream:
    """State for one of two interleaved GRU scan directions.

    rev_base: None -> xg fetched at storage idx (off + iv*U + u);
              int  -> fetched at (rev_base - (off + iv*U + u)).
    flush_rev: owned h stored descending into hT_out's t axis.
    """

    def __init__(self, tc, name, ctx, w_src, bhn_src, zpad_src, xg_v,
                 hT_out, rev_base, flush_rev, n_steps, flush_lo, zeros_bf,
                 U=16):
        nc = tc.nc
        self.tc = tc
        self.name = name
        self.rev_base = rev_base
        self.flush_rev = flush_rev
        self.n_steps = n_steps
        self.flush_lo = flush_lo
        self.hT_out = hT_out            # [D, n_out*B] flat, ascending t
        self.n_out = hT_out.shape[1] // B
        self.U = U
        wp = ctx.enter_context(tc.tile_pool(name=f"w_{name}", bufs=1))
        st = ctx.enter_context(tc.tile_pool(name=f"s_{name}", bufs=1))
        self.pool = ctx.enter_context(tc.tile_pool(name=f"t_{name}", bufs=2))
        self.slabp = ctx.enter_context(
            tc.tile_pool(name=f"sl_{name}", bufs=2))
        self.pp = ctx.enter_context(
            tc.tile_pool(name=f"p_{name}", bufs=1, space="PSUM"))
        self.ppt = ctx.enter_context(
            tc.tile_pool(name=f"pt_{name}", bufs=1, space="PSUM"))

        self.w_sb = wp.tile([128, KD * H3], BF16, name="w_sb")
        nc.sync.dma_start(self.w_sb[:], w_src[:, :])
        self.bhn = wp.tile([128, 256], F32, name="bhn")
        nc.sync.dma_start(self.bhn[:], bhn_src[:, :])
        self.zpad = wp.tile([128, n_steps], F32, name="zpad")
        nc.sync.dma_start(self.zpad[:], zpad_src[:, 0:n_steps])

        self.hgrp = st.tile([128, 256], F32, name="hgrp")
        nc.gpsimd.memset(self.hgrp[:], 0.0)
        self.hT_hist = st.tile([128, U * 64], BF16, name="hT_hist")
        nc.sync.dma_start(self.hT_hist[:], zeros_bf[:, 0:U * 64])
        self.xg_t = xg_v        # [nblk, G, B, 16, 768] bf16
        self.slab = None

    def step(self, iv, u, off, zeros_st, zrhs, ident):
        nc = self.tc.nc
        pool, pp, ppt = self.pool, self.pp, self.ppt
        U = self.U
        rev = self.rev_base is not None
        slot = (U - 1 - u) if rev else u
        pslot = (slot + 1) % U if rev else (slot - 1) % U
        if not rev:
            t_el = iv * U + (u + off)
        else:
            t_el = iv * (-U) + (self.rev_base - u - off)
        if u % 8 == 0:
            # fetch xg for the next 8 scan steps into an SBUF slab
            self.slab = self.slabp.tile([128, 8 * 768], BF16, name="slab")
            if not rev:
                st0c = u + off              # storage start = iv*U + st0c
                blk = iv + st0c // 16
                hb = (st0c % 16) // 8
            else:
                st0c = self.rev_base - 7 - u - off
                blk = iv * (-1) + st0c // 16
                hb = (st0c % 16) // 8
            for j in range(G):
                srcj = self.xg_t[ds(blk, 1), j, :, hb * 8:hb * 8 + 8, :]
                srcj = srcj.rearrange("a b u c -> (a b) (u c)")
                nc.sync.dma_start(self.slab[32 * j:32 * j + B, :], srcj)
        sc = ((7 - u % 8) if rev else (u % 8)) * 768
        xgt = self.slab[:, sc:sc + 768]

        gates = pp.tile([128, 768], F32, name="gates")
        nc.tensor.matmul(gates[:, 0:512], zeros_st[:], zrhs[:],
                         start=True, stop=False)
        nc.tensor.matmul(gates[:, 512:768], zeros_st[:], zrhs[:, 0:256],
                         start=True, stop=False)
        for k in range(KD):
            kk = (k % 2) * 4 + k // 2
            lof = kk * 128 + pslot * 8
            lhsT = self.hT_hist[:, lof:lof + 8]
            for j in range(G):
                wof = k * H3 + j * 768
                nc.tensor.matmul(gates[32 * j:32 * j + 8, 0:512], lhsT,
                                 self.w_sb[:, wof:wof + 512],
                                 start=False, stop=False,
                                 tile_position=(0, 32 * j))
                nc.tensor.matmul(gates[32 * j:32 * j + 8, 512:768], lhsT,
                                 self.w_sb[:, wof + 512:wof + 768],
                                 start=False, stop=(k == KD - 1),
                                 tile_position=(0, 32 * j))

        grz = pool.tile([128, 512], F32, name="grz")
        nc.vector.tensor_add(grz[:NP, 0:256], gates[:NP, 0:256],
                             xgt[:NP, 0:256])
        nc.vector.scalar_tensor_tensor(
            grz[:NP, 256:512], gates[:NP, 256:512],
            self.zpad[:NP, ds(iv * U + u + off, 1)], xgt[:NP, 256:512],
            op0=ALU.add, op1=ALU.add)
        rz = pool.tile([128, 512], F32, name="rz")
        nc.scalar.activation(rz[:NP], grz[:NP], AF.Sigmoid)
        t2a = pool.tile([128, 256], F32, name="t2a")
        nc.vector.tensor_add(t2a[:NP], gates[:NP, 512:768], self.bhn[:NP])
        t2 = pool.tile([128, 256], F32, name="t2")
        nc.vector.tensor_mul(t2[:NP], rz[:NP, 0:256], t2a[:NP])
        npre = pool.tile([128, 256], F32, name="npre")
        nc.vector.tensor_add(npre[:NP], t2[:NP], xgt[:NP, 512:768])
        nn = pool.tile([128, 256], F32, name="nn")
        nc.scalar.activation(nn[:NP], npre[:NP], AF.Tanh)
        dlt = pool.tile([128, 256], F32, name="dlt")
        nc.vector.tensor_sub(dlt[:NP], self.hgrp[:NP], nn[:NP])
        e = pool.tile([128, 256], F32, name="e")
        nc.vector.tensor_mul(e[:NP], rz[:NP, 256:512], dlt[:NP])
        nc.vector.tensor_add(self.hgrp[:NP], nn[:NP], e[:NP])

        tp = ppt.tile([128, 256], F32, name="tp")
        for cc in range(2):
            nc.tensor.transpose(tp[:, 128 * cc:128 * cc + NP],
                                self.hgrp[0:NP, 128 * cc:128 * (cc + 1)],
                                ident[0:NP, 0:NP])
        tp4 = tp.rearrange("p (c j r) -> p (c j) r", c=2, j=G)[:, :, 0:B]
        hist4 = self.hT_hist.rearrange("p (kk s b) -> p kk s b",
                                       kk=8, s=U)
        nc.scalar.activation(hist4[:, :, slot, :], tp4, AF.Copy)

    def flush(self, iv):
        """Flush h.T for scan steps s = flush_lo + iv*U + [0,U) to
        hT_out t-idx (s-flush_lo) ascending, or n_out-1-(s-flush_lo)
        descending when flush_rev."""
        nc = self.tc.nc
        U = self.U
        UB = U * B
        for kk in range(KD):
            k = (kk % 4) * 2 + kk // 4
            src = self.hT_hist[:, kk * 128:(kk + 1) * 128]
            if self.flush_rev:
                dst = self.hT_out[k * 128:(k + 1) * 128,
                                  ds(iv * (-UB) + (self.n_out - U) * B, UB)]
            else:
                dst = self.hT_out[k * 128:(k + 1) * 128, ds(iv * UB, UB)]
            nc.sync.dma_start(dst, src)


def build_scan_pair(tc, specs, zeros_st, zrhs, ident, zeros_bf):
    nc = tc.nc
    U = 16
    with contextlib.ExitStack() as c:
        streams = [ScanStream(tc, sp["name"], c, sp["w"], sp["bhn"],
                              sp["zpad"], sp["xg"], sp["hT"],
                              sp["rev_base"], sp["flush_rev"],
                              sp["n_steps"], sp["flush_lo"], zeros_bf, U=U)
                   for sp in specs]
        n_steps = specs[0]["n_steps"]
        flush_lo = specs[0]["flush_lo"]
        assert all(sp["n_steps"] == n_steps and sp["flush_lo"] == flush_lo
                   for sp in specs)
        assert flush_lo % U == 0 and n_steps % U == 0
        nf = flush_lo // U
        if nf > 0:
            with tc.For_i(0, nf) as iv:
                for u in range(U):
                    for s in streams:
                        s.step(iv, u, 0, zeros_st, zrhs, ident)
        with tc.For_i(0, (n_steps - flush_lo) // U) as iv:
            for u in range(U):
                for s in streams:
                    s.step(iv, u, flush_lo, zeros_st, zrhs, ident)
            for s in streams:
                s.flush(iv)


def build_proj(tc, dram, zeros_st, zrhs, ident, nt):
    """x2 = x_own + concat(h1A,h1B) @ gru_out.T; x2nT for FFN."""
    nc = tc.nc
    h1a = dram["hT1_A"]
    h1b = dram["hT1_B"]
    own0 = 2 * W * B
    with contextlib.ExitStack() as c:
        wp = c.enter_context(tc.tile_pool(name="pj_w", bufs=1))
        pool = c.enter_context(tc.tile_pool(name="pj_t", bufs=3))
        stp = c.enter_context(tc.tile_pool(name="pj_s", bufs=2))
        pp = c.enter_context(tc.tile_pool(name="pj_p", bufs=4, space="PSUM"))

        gw = wp.tile([128, 2 * KD * D], BF16, name="gw")
        for k in range(2 * KD):
            nc.sync.dma_start(gw[:, k * D:(k + 1) * D],
                              dram["gru_wT"][k * 128:(k + 1) * 128, :])

        with tc.For_i(0, nt) as tv:
            tok = tv * 128
            sts = []
            for k in range(2 * KD):
                stt = stp.tile([128, 128], BF16, name=f"pst{k}")
                srcv = h1a if k < KD else h1b
                kk = k % KD
                nc.sync.dma_start(
                    stt[:], srcv[kk * 128:(kk + 1) * 128, ds(tok, 128)])
                sts.append(stt)
            x2 = pool.tile([128, D], F32, name="x2")
            for cc in range(2):
                ps = pp.tile([128, 512], F32, name="ps")
                nc.tensor.matmul(ps[:], zeros_st[:], zrhs[:],
                                 start=True, stop=False)
                for k in range(2 * KD):
                    nc.tensor.matmul(
                        ps[:], sts[k][:],
                        gw[:, k * D + 512 * cc:k * D + 512 * cc + 512],
                        start=False, stop=(k == 2 * KD - 1))
                xt = pool.tile([128, 512], F32, name="xt")
                nc.sync.dma_start(
                    xt[:], dram["x_win"][ds(tok + own0, 128),
                                         512 * cc:512 * cc + 512])
                nc.vector.tensor_add(x2[:, 512 * cc:512 * cc + 512],
                                     ps[:], xt[:])
            nc.sync.dma_start(dram["x2"][ds(tok, 128), :], x2[:])
            sq = pool.tile([128, D], F32, name="sq")
            ssum = pool.tile([128, 1], F32, name="ssum")
            nc.scalar.activation(sq[:], x2[:], AF.Square, accum_out=ssum[:])
            m = pool.tile([128, 1], F32, name="m")
            nc.vector.tensor_scalar(m[:], ssum[:], 1.0 / D, EPS,
                                    op0=ALU.mult, op1=ALU.add)
            r = pool.tile([128, 1], F32, name="r")
            nc.vector.reciprocal(r[:], m[:])
            s2 = pool.tile([128, 1], F32, name="s2")
            nc.scalar.activation(s2[:], r[:], AF.Sqrt)
            x2n = pool.tile([128, D], F32, name="x2n")
            nc.vector.tensor_scalar_mul(x2n[:], x2[:], s2[:])
            for k in range(KD):
                tpp = pp.tile([128, 128], F32, name="tpp")
                nc.tensor.transpose(tpp[:], x2n[:, k * 128:(k + 1) * 128],
                                    ident[:])
                xc = pool.tile([128, 128], F32R, name="xc")
                nc.scalar.activation(xc[:], tpp[:], AF.Copy)
                nc.sync.dma_start(
                    dram["x2nT"][k * 128:(k + 1) * 128, ds(tok, 128)],
                    xc[:])


def build_ffn13(tc, dram, zeros_st, zrhs, ident, nt):
    nc = tc.nc
    with contextlib.ExitStack() as c:
        wp = c.enter_context(tc.tile_pool(name="fb_w", bufs=1))
        pool = c.enter_context(tc.tile_pool(name="fb_t", bufs=3))
        stp = c.enter_context(tc.tile_pool(name="fb_s", bufs=2))
        pp = c.enter_context(tc.tile_pool(name="fb_p", bufs=2, space="PSUM"))

        w1 = wp.tile([128, KD * FFN], F32R, name="w1")
        w3 = wp.tile([128, KD * FFN], F32R, name="w3")
        for k in range(KD):
            nc.sync.dma_start(w1[:, k * FFN:(k + 1) * FFN],
                              dram["w1T"][k * 128:(k + 1) * 128, :])
            nc.sync.dma_start(w3[:, k * FFN:(k + 1) * FFN],
                              dram["w3T"][k * 128:(k + 1) * 128, :])

        FCH = [(c0, min(512, FFN - c0)) for c0 in range(0, FFN, 512)]
        with tc.For_i(0, nt) as tv:
            tok = tv * 128
            sts = []
            for k in range(KD):
                stt = stp.tile([128, 128], F32R, name=f"bst{k}")
                nc.sync.dma_start(
                    stt[:], dram["x2nT"][k * 128:(k + 1) * 128, ds(tok, 128)])
                sts.append(stt)
            for (c0, cn) in FCH:
                p1 = pp.tile([128, 512], F32, name="p1")
                p3 = pp.tile([128, 512], F32, name="p3")
                nc.tensor.matmul(p1[:, :cn], zeros_st[:], zrhs[:, :cn],
                                 start=True, stop=False)
                nc.tensor.matmul(p3[:, :cn], zeros_st[:], zrhs[:, :cn],
                                 start=True, stop=False)
                for k in range(KD):
                    nc.tensor.matmul(p1[:, :cn], sts[k][:],
                                     w1[:, k * FFN + c0:k * FFN + c0 + cn],
                                     start=False, stop=(k == KD - 1))
                    nc.tensor.matmul(p3[:, :cn], sts[k][:],
                                     w3[:, k * FFN + c0:k * FFN + c0 + cn],
                                     start=False, stop=(k == KD - 1))
                sl = pool.tile([128, 512], F32, name="sl")
                nc.scalar.activation(sl[:, :cn], p1[:, :cn], AF.Silu)
                h1c = pool.tile([128, 512], F32, name="h1c")
                nc.vector.tensor_mul(h1c[:, :cn], sl[:, :cn], p3[:, :cn])
                for q in range(cn // 128):
                    tpp = pp.tile([128, 128], F32, name="tpp")
                    nc.tensor.transpose(
                        tpp[:], h1c[:, q * 128:(q + 1) * 128], ident[:])
                    hc = pool.tile([128, 128], F32R, name="hc")
                    nc.scalar.activation(hc[:], tpp[:], AF.Copy)
                    kf = (c0 + q * 128) // 128
                    nc.sync.dma_start(
                        dram["h1T"][kf * 128:(kf + 1) * 128, ds(tok, 128)],
                        hc[:])


def build_ffn2(tc, dram, zeros_st, zrhs, nt):
    nc = tc.nc
    with contextlib.ExitStack() as c:
        wp = c.enter_context(tc.tile_pool(name="fc_w", bufs=1))
        pool = c.enter_context(tc.tile_pool(name="fc_t", bufs=3))
        stp = c.enter_context(tc.tile_pool(name="fc_s", bufs=2))
        pp = c.enter_context(tc.tile_pool(name="fc_p", bufs=4, space="PSUM"))

        w2 = wp.tile([128, KF * D], F32R, name="w2")
        for k in range(KF):
            nc.sync.dma_start(w2[:, k * D:(k + 1) * D],
                              dram["w2T"][k * 128:(k + 1) * 128, :])

        with tc.For_i(0, nt) as tv:
            tok = tv * 128
            sts = []
            for k in range(KF):
                stt = stp.tile([128, 128], F32R, name=f"cst{k}")
                nc.sync.dma_start(
                    stt[:],
                    dram["h1T"][k * 128:(k + 1) * 128, ds(tok, 128)])
                sts.append(stt)
            for cc in range(2):
                ps = pp.tile([128, 512], F32, name="ps")
                nc.tensor.matmul(ps[:], zeros_st[:], zrhs[:],
                                 start=True, stop=False)
                for k in range(KF):
                    nc.tensor.matmul(
                        ps[:], sts[k][:],
                        w2[:, k * D + 512 * cc:k * D + 512 * cc + 512],
                        start=False, stop=(k == KF - 1))
                xt = pool.tile([128, 512], F32, name="xt")
                nc.sync.dma_start(
                    xt[:], dram["x2"][ds(tok, 128),
                                      512 * cc:512 * cc + 512])
                yo = pool.tile([128, 512], F32, name="yo")
                nc.vector.tensor_add(yo[:], ps[:], xt[:])
                nc.sync.dma_start(
                    dram["y"][ds(tok, 128), 512 * cc:512 * cc + 512],
                    yo[:])


def build_program(nc):
    dram = {}

    def din(name, shape, dt=F32R):
        dram[name] = nc.dram_tensor(name, shape, dt, kind="ExternalInput").ap()

    def dout(name, shape, dt=F32):
        dram[name] = nc.dram_tensor(name, shape, dt,
                                    kind="ExternalOutput").ap()

    def dtmp(name, shape, dt=F32R):
        dram[name] = nc.dram_tensor(name, shape, dt).ap()

    din("x_win", [T0X * B, D], F32)
    din("x_winT", [D, T0X * B], BF16)
    for ss in ("A", "B"):
        din(f"wA_{ss}", [D, H3], BF16)
        din(f"biasA_{ss}", [128, H3], F32)
        din(f"wD_{ss}", [2 * D, H3], BF16)
        din(f"biasD_{ss}", [128, H3], F32)
        din(f"wS0_{ss}", [128, KD * H3], BF16)
        din(f"bhn0_{ss}", [128, 256], F32)
        din(f"wS1_{ss}", [128, KD * H3], BF16)
        din(f"bhn1_{ss}", [128, 256], F32)
        din(f"zpad0_{ss}", [128, T0], F32)
        din(f"zpad1_{ss}", [128, T1], F32)
    din("zeros", [128, 1024])
    din("zeros_bf", [128, 1024], BF16)
    din("gru_wT", [2 * D, D], BF16)
    din("w1T", [D, FFN])
    din("w3T", [D, FFN])
    din("w2T", [FFN, D])
    dout("y", [L * B, D])

    for ss in ("A", "B"):
        dtmp(f"xg0_{ss}", [T0X // 16, G, B, 16, 768], BF16)
        dtmp(f"xg1_{ss}", [T1 // 16, G, B, 16, 768], BF16)
        dtmp(f"hT0_{ss}", [D, T0H * B], BF16)
        dtmp(f"hT1_{ss}", [D, L * B], BF16)
    dtmp("x2", [L * B, D], F32)
    dtmp("x2nT", [D, L * B])
    dtmp("h1T", [FFN, L * B])

    NT0 = T0X * B // 128       # 16
    NT1 = T1 * B // 128        # 10
    NTP = L * B // 128         # 8

    with tile.TileContext(nc) as tc:
        with tc.tile_pool(name="consts", bufs=1) as consts:
            zeros_st = consts.tile([1, 128], F32R, name="zeros_st")
            nc.sync.dma_start(zeros_st[:], dram["zeros"][0:1, 0:128])
            zrhs = consts.tile([1, 512], F32R, name="zrhs")
            nc.sync.dma_start(zrhs[:], dram["zeros"][0:1, 0:512])
            ident = consts.tile([128, 128], F32, name="ident")
            make_identity(nc, ident[:])
            s_sb = consts.tile([128, NT0], F32, name="s_sb")

            with nc.named_scope("norm"):
                build_norm_stats(tc, dram["x_win"], s_sb, NT0)
            xt_views = [dram["x_winT"][k * 128:(k + 1) * 128, :]
                        for k in range(KD)]
            with nc.named_scope("xg0"):
                build_xg_gemm(tc, xt_views, KD,
                              [dram["wA_A"], dram["wA_B"]],
                              [dram["biasA_A"], dram["biasA_B"]], s_sb,
                              [dram["xg0_A"], dram["xg0_B"]],
                              zeros_st, zrhs, NT0)
            with nc.named_scope("scan0"):
                build_scan_pair(
                    tc,
                    [dict(name="s0A", w=dram["wS0_A"], bhn=dram["bhn0_A"],
                          zpad=dram["zpad0_A"], xg=dram["xg0_A"],
                          hT=dram["hT0_A"], rev_base=None, flush_rev=False,
                          n_steps=T0, flush_lo=W),
                     dict(name="s0B", w=dram["wS0_B"], bhn=dram["bhn0_B"],
                          zpad=dram["zpad0_B"], xg=dram["xg0_B"],
                          hT=dram["hT0_B"], rev_base=T0X - 1,
                          flush_rev=True, n_steps=T0, flush_lo=W)],
                    zeros_st, zrhs, ident, dram["zeros_bf"])
            # xg1_A covers h0 idx [0,T1) ascending; xg1_B covers idx
            # [W,T0H) ascending (its scan fetches it reversed).
            h0a = dram["hT0_A"]
            h0b = dram["hT0_B"]
            for ss, off in (("A", 0), ("B", W * B)):
                sv = [h0a[k * 128:(k + 1) * 128, off:off + T1 * B]
                      for k in range(KD)]
                sv += [h0b[k * 128:(k + 1) * 128, off:off + T1 * B]
                       for k in range(KD)]
                with nc.named_scope(f"xg1{ss}"):
                    build_xg_gemm(tc, sv, 2 * KD, [dram[f"wD_{ss}"]],
                                  [dram[f"biasD_{ss}"]], None,
                                  [dram[f"xg1_{ss}"]],
                                  zeros_st, zrhs, NT1, wdt=BF16, sdt=BF16)
            with nc.named_scope("scan1"):
                build_scan_pair(
                    tc,
                    [dict(name="s1A", w=dram["wS1_A"], bhn=dram["bhn1_A"],
                          zpad=dram["zpad1_A"], xg=dram["xg1_A"],
                          hT=dram["hT1_A"], rev_base=None, flush_rev=False,
                          n_steps=T1, flush_lo=W),
                     dict(name="s1B", w=dram["wS1_B"], bhn=dram["bhn1_B"],
                          zpad=dram["zpad1_B"], xg=dram["xg1_B"],
                          hT=dram["hT1_B"], rev_base=T1 - 1,
                          flush_rev=True, n_steps=T1, flush_lo=W)],
                    zeros_st, zrhs, ident, dram["zeros_bf"])
            with nc.named_scope("proj"):
                build_proj(tc, dram, zeros_st, zrhs, ident, NTP)
            with nc.named_scope("ffn13"):
                build_ffn13(tc, dram, zeros_st, zrhs, ident, NTP)
            with nc.named_scope("ffn2"):
                build_ffn2(tc, dram, zeros_st, zrhs, NTP)
    return dram


# ================================================================== driver
_CACHE = {}


def _prep_shared(inputs):
    import ml_dtypes
    bf = ml_dtypes.bfloat16
    gnw = np.asarray(inputs["gru_norm_w"], np.float32)
    fnw = np.asarray(inputs["ffn_norm_w"], np.float32)
    sh = {}
    for di, ss in ((0, "A"), (1, "B")):
        sh[f"wA_{ss}"] = prep_gemm_weights(
            np.asarray(inputs["w_ih_l0"], np.float32)[di], gnw).astype(bf)
        sh[f"biasA_{ss}"] = prep_gemm_bias(
            np.asarray(inputs["b_ih_l0"], np.float32)[di],
            np.asarray(inputs["b_hh_l0"], np.float32)[di])
        sh[f"wD_{ss}"] = prep_gemm_weights(
            np.asarray(inputs["w_ih_l1"], np.float32)[di]).astype(bf)
        sh[f"biasD_{ss}"] = prep_gemm_bias(
            np.asarray(inputs["b_ih_l1"], np.float32)[di],
            np.asarray(inputs["b_hh_l1"], np.float32)[di])
        for Lx in (0, 1):
            sh[f"wS{Lx}_{ss}"] = prep_scan_weights(
                np.asarray(inputs[f"w_hh_l{Lx}"], np.float32)[di]).astype(bf)
            sh[f"bhn{Lx}_{ss}"] = prep_bhn_scan(
                np.asarray(inputs[f"b_hh_l{Lx}"], np.float32)[di])
    sh["zeros"] = np.zeros((128, 1024), np.float32)
    sh["zeros_bf"] = np.zeros((128, 1024), bf)
    sh["gru_wT"] = np.ascontiguousarray(
        np.asarray(inputs["gru_out_w"], np.float32).T).astype(bf)
    sh["w1T"] = np.ascontiguousarray(
        (np.asarray(inputs["w1"], np.float32) * fnw[None, :]).T)
    sh["w3T"] = np.ascontiguousarray(
        (np.asarray(inputs["w3"], np.float32) * fnw[None, :]).T)
    sh["w2T"] = np.ascontiguousarray(np.asarray(inputs["w2"], np.float32).T)
    return sh


def _host_inputs_spmd(inputs):
    x = np.asarray(inputs["x"], np.float32)          # [B, S, D]
    sh = _prep_shared(inputs)
    im_list = []
    for c in range(NCORE):
        im = dict(sh)
        lo = c * L - 2 * W
        idx = np.arange(lo, lo + T0X)
        valid = (idx >= 0) & (idx < S)
        xw = np.zeros((T0X, B, D), np.float32)
        xw[valid] = x[:, idx[valid], :].transpose(1, 0, 2)
        xw = xw.reshape(T0X * B, D)
        im["x_win"] = np.ascontiguousarray(xw)
        import ml_dtypes
        im["x_winT"] = np.ascontiguousarray(xw.T).astype(ml_dtypes.bfloat16)
        for ss, rev in (("A", False), ("B", True)):
            sidx = np.arange(T0)
            widx = sidx if not rev else (T0X - 1 - sidx)
            t = lo + widx
            z0 = np.zeros((128, T0), np.float32)
            z0[:] = (((t < 0) | (t >= S)) * 40.0)[None, :]
            im[f"zpad0_{ss}"] = z0
            s1 = np.arange(T1)
            hidx = s1 if not rev else (T0H - 1 - s1)
            t1 = (lo + W) + hidx
            z1 = np.zeros((128, T1), np.float32)
            z1[:] = (((t1 < 0) | (t1 >= S)) * 40.0)[None, :]
            im[f"zpad1_{ss}"] = z1
        im_list.append(im)
    return im_list


def get_compiled(n_cores=NCORE):
    if "nc" not in _CACHE:
        nc = bacc.Bacc("TRN2", target_bir_lowering=False, debug=False,
                       num_devices=n_cores)
        build_program(nc)
        nc.compile()
        _CACHE["nc"] = nc
        _CACHE["n_cores"] = n_cores
    return _CACHE["nc"], _CACHE["n_cores"]


def kernel(**inputs) -> np.ndarray:
    im_list = _host_inputs_spmd(inputs)
    nc, n_cores = get_compiled()
    res = run_bass_kernel_spmd(nc, im_list, core_ids=list(range(n_cores)))
    y = np.zeros((B, S, D), np.float32)
    for c in range(n_cores):
        yc = res.results[c]["y"].reshape(L, B, D)
        y[:, c * L:(c + 1) * L, :] = yc.transpose(1, 0, 2)
    return y


# revision 7
# speedup vs baseline: 2.0995x; 1.1356x over previous
"""Trainium2 Bass kernel for nn_BidirectionalGRU (B=8,S=1024,D=1024), v2.

Sharding: 8 cores = 8 time-chunks of L=128 tokens. Every core runs BOTH
GRU directions (streams A=fwd, B=bwd) over its own chunk window with
warm-up margins, so all cross-chunk dependence is absorbed by warm-up
(GRU state contracts; W=32 gives ~1e-7 end-to-end error on CPU) and no
collectives are needed. Per-core differences are input data only
(x windows, z-gate pad masks); the program is SPMD-uniform.

Windows (chunk c, warm-up W, lo = 128c-2W):
  x window:   t in [lo, lo+T0X), T0X=L+4W, stored ascending.
  L0 scan:    A ascends window idx from 0; B descends from T0X-1;
              T0=L+3W steps; h0 flushed for steps>=W into hT0[D,T0H,B]
              stored ascending-t (B's flush reversed), T0H=L+2W,
              covering t in [lo+W, lo+W+T0H).
  xg1:        stream A tokens = h0 idx [0,T1) asc; B = idx [T0H-1..W]
              desc (= its scan order); T1=L+W.
  L1 scan:    both fetch xg1 ascending (already scan-ordered); flush
              steps>=W into hT1[D,L,B] ascending-t (B reversed).
  Out-of-range steps (edge cores) are neutralized by zpad=+40 on the
  z-gate pre-activation (z=1 freezes h=0 exactly); host pads x with 0.
"""
import contextlib
import numpy as np

import concourse.bacc as bacc
import concourse.tile as tile
from concourse import mybir
from concourse.bass import ds
from concourse.bass_utils import run_bass_kernel_spmd
from concourse.masks import make_identity

F32 = mybir.dt.float32
F32R = mybir.dt.float32r
BF16 = mybir.dt.bfloat16
AF = mybir.ActivationFunctionType
ALU = mybir.AluOpType

B, S, D, H3, G, FFN = 8, 1024, 1024, 3072, 4, 2816
NCORE = 8
L = S // NCORE               # 128 owned tokens per core
W = 16                       # warm-up steps
T0X = L + 4 * W              # x window tokens (256)
T0 = L + 3 * W               # L0 scan steps (224)
T0H = L + 2 * W              # h0 stored tokens (192)
T1 = L + W                   # L1 scan steps (160)
KD = D // 128                # 8
KF = FFN // 128              # 22
EPS = 1e-5
NP = 104                     # partitions spanned by grouped layout


# ================================================================ host prep
def gate_perm():
    idx = []
    for j in range(G):
        for blk in range(3):
            base = blk * 1024 + j * 256
            idx.extend(range(base, base + 256))
    return np.array(idx)

PERM = gate_perm()


def prep_scan_weights(w_hh_d):
    wp = w_hh_d[PERM]
    wt = wp.T.reshape(KD, 128, H3).transpose(1, 0, 2)
    return np.ascontiguousarray(wt.reshape(128, KD * H3), dtype=np.float32)


def prep_gemm_weights(w_ih_d, norm_w=None):
    wp = w_ih_d[PERM]
    if norm_w is not None:
        wp = wp * norm_w[None, :]
    return np.ascontiguousarray(wp.T, dtype=np.float32)


def prep_gemm_bias(b_ih_d, b_hh_d):
    bi = b_ih_d[PERM].copy()
    bh = b_hh_d[PERM]
    m = np.where(np.arange(H3) % 768 < 512, bh, 0.0)
    b = (bi + m).astype(np.float32)
    return np.ascontiguousarray(np.broadcast_to(b, (128, H3)), dtype=np.float32)


def prep_bhn_scan(b_hh_d):
    bh = b_hh_d[PERM].reshape(G, 3, 256)[:, 2, :]
    out = np.zeros((128, 256), np.float32)
    for j in range(G):
        out[32 * j:32 * j + 32, :] = bh[j][None, :]
    return out


# ============================================================ device builders
def build_norm_stats(tc, x_nat, s_sb, nt):
    nc = tc.nc
    with tc.tile_pool(name="nstat", bufs=3) as pool:
        for i in range(nt):
            xt = pool.tile([128, D], F32, name="xt")
            nc.sync.dma_start(xt[:], x_nat[i * 128:(i + 1) * 128, :])
            sq = pool.tile([128, D], F32, name="sq")
            ss = pool.tile([128, 1], F32, name="ss")
            nc.scalar.activation(sq[:], xt[:], AF.Square, accum_out=ss[:])
            m = pool.tile([128, 1], F32, name="m")
            nc.vector.tensor_scalar(m[:], ss[:], 1.0 / D, EPS,
                                    op0=ALU.mult, op1=ALU.add)
            r = pool.tile([128, 1], F32, name="r")
            nc.vector.reciprocal(r[:], m[:])
            nc.scalar.activation(s_sb[:, i:i + 1], r[:], AF.Sqrt)


def build_xg_gemm(tc, stat_views, n_k, ws, biases, s_sb, out_vs,
                  zeros_st, zrhs, nt, wdt=BF16, sdt=BF16):
    """out[blk, g, b, u, 768] = s*(x @ w) + bias for 1-2 streams.
    Weights AND stationary token tiles are SBUF-resident (full unroll:
    matmul stationary APs cannot take register offsets)."""
    nc = tc.nc
    ns = len(ws)
    ncols = nt * 128
    with contextlib.ExitStack() as c:
        wp = c.enter_context(tc.tile_pool(name="xg_w", bufs=1))
        pool = c.enter_context(tc.tile_pool(name="xg_t", bufs=3))
        pp = c.enter_context(tc.tile_pool(name="xg_p", bufs=4, space="PSUM"))

        bias_sb = wp.tile([128, ns * H3], F32, name="bias_sb")
        wsb = wp.tile([128, ns * n_k * H3], wdt, name="wsb")
        for si in range(ns):
            nc.sync.dma_start(bias_sb[:, si * H3:(si + 1) * H3],
                              biases[si][:, :])
            for k in range(n_k):
                nc.sync.dma_start(
                    wsb[:, (si * n_k + k) * H3:(si * n_k + k + 1) * H3],
                    ws[si][k * 128:(k + 1) * 128, :])
        stsb = wp.tile([128, n_k * ncols], sdt, name="stsb")
        for k in range(n_k):
            nc.sync.dma_start(stsb[:, k * ncols:(k + 1) * ncols],
                              stat_views[k][:, :])
        for tv in range(nt):
            tok = tv * 128
            for si in range(ns):
                for c0 in range(0, H3, 512):
                    ps = pp.tile([128, 512], F32, name="ps")
                    nc.tensor.matmul(ps[:], zeros_st[:], zrhs[:],
                                     start=True, stop=False)
                    for k in range(n_k):
                        wof = (si * n_k + k) * H3 + c0
                        nc.tensor.matmul(
                            ps[:], stsb[:, k * ncols + tok:
                                        k * ncols + tok + 128],
                            wsb[:, wof:wof + 512],
                            start=False, stop=(k == n_k - 1))
                    o = pool.tile([128, 512], BF16, name="o")
                    if s_sb is not None:
                        nc.vector.scalar_tensor_tensor(
                            o[:], ps[:], s_sb[:, tv:tv + 1],
                            bias_sb[:, si * H3 + c0:si * H3 + c0 + 512],
                            op0=ALU.mult, op1=ALU.add)
                    else:
                        nc.vector.tensor_add(
                            o[:], ps[:],
                            bias_sb[:, si * H3 + c0:si * H3 + c0 + 512])
                    cc = c0
                    while cc < c0 + 512:
                        g, gc = divmod(cc, 768)
                        take = min(768 - gc, c0 + 512 - cc)
                        dstv = out_vs[si][tv, g, :, :, gc:gc + take]
                        dstv = dstv.rearrange("b u c -> u b c")
                        nc.sync.dma_start(
                            dstv, o[:, cc - c0:cc - c0 + take])
                        cc += take


class ScanStream:
    """State for one of two interleaved GRU scan directions.

    rev_base: None -> xg fetched at storage idx (off + iv*U + u);
              int  -> fetched at (rev_base - (off + iv*U + u)).
    flush_rev: owned h stored descending into hT_out's t axis.
    """

    def __init__(self, tc, name, ctx, w_src, bhn_src, zpad_src, xg_v,
                 hT_out, rev_base, flush_rev, n_steps, flush_lo, zeros_bf,
                 U=16):
        nc = tc.nc
        self.tc = tc
        self.name = name
        self.rev_base = rev_base
        self.flush_rev = flush_rev
        self.n_steps = n_steps
        self.flush_lo = flush_lo
        self.hT_out = hT_out            # [D, n_out*B] flat, ascending t
        self.n_out = hT_out.shape[1] // B
        self.U = U
        wp = ctx.enter_context(tc.tile_pool(name=f"w_{name}", bufs=1))
        st = ctx.enter_context(tc.tile_pool(name=f"s_{name}", bufs=1))
        self.pool = ctx.enter_context(tc.tile_pool(name=f"t_{name}", bufs=2))
        self.slabp = ctx.enter_context(
            tc.tile_pool(name=f"sl_{name}", bufs=2))
        self.pp = ctx.enter_context(
            tc.tile_pool(name=f"p_{name}", bufs=1, space="PSUM"))
        self.ppt = ctx.enter_context(
            tc.tile_pool(name=f"pt_{name}", bufs=1, space="PSUM"))

        self.w_sb = wp.tile([128, KD * H3], BF16, name="w_sb")
        nc.sync.dma_start(self.w_sb[:], w_src[:, :])
        self.bhn = wp.tile([128, 256], F32, name="bhn")
        nc.sync.dma_start(self.bhn[:], bhn_src[:, :])
        self.zpad = wp.tile([128, n_steps], F32, name="zpad")
        nc.sync.dma_start(self.zpad[:], zpad_src[:, 0:n_steps])

        self.hgrp = st.tile([128, 256], F32, name="hgrp")
        nc.gpsimd.memset(self.hgrp[:], 0.0)
        self.hT_hist = st.tile([128, U * 64], BF16, name="hT_hist")
        nc.sync.dma_start(self.hT_hist[:], zeros_bf[:, 0:U * 64])
        self.xg_t = xg_v        # [nblk, G, B, 16, 768] bf16
        self.slab = None

    def step(self, iv, u, off, zeros_st, zrhs, ident):
        nc = self.tc.nc
        pool, pp, ppt = self.pool, self.pp, self.ppt
        U = self.U
        rev = self.rev_base is not None
        slot = (U - 1 - u) if rev else u
        pslot = (slot + 1) % U if rev else (slot - 1) % U
        if not rev:
            t_el = iv * U + (u + off)
        else:
            t_el = iv * (-U) + (self.rev_base - u - off)
        if u % 8 == 0:
            # fetch xg for the next 8 scan steps into an SBUF slab
            self.slab = self.slabp.tile([128, 8 * 768], BF16, name="slab")
            if not rev:
                st0c = u + off              # storage start = iv*U + st0c
                blk = iv + st0c // 16
                hb = (st0c % 16) // 8
            else:
                st0c = self.rev_base - 7 - u - off
                blk = iv * (-1) + st0c // 16
                hb = (st0c % 16) // 8
            for j in range(G):
                srcj = self.xg_t[ds(blk, 1), j, :, hb * 8:hb * 8 + 8, :]
                srcj = srcj.rearrange("a b u c -> (a b) (u c)")
                nc.sync.dma_start(self.slab[32 * j:32 * j + B, :], srcj)
        sc = ((7 - u % 8) if rev else (u % 8)) * 768
        xgt = self.slab[:, sc:sc + 768]

        gates = pp.tile([128, 768], F32, name="gates")
        nc.tensor.matmul(gates[:, 0:512], zeros_st[:], zrhs[:],
                         start=True, stop=False)
        nc.tensor.matmul(gates[:, 512:768], zeros_st[:], zrhs[:, 0:256],
                         start=True, stop=False)
        for k in range(KD):
            kk = (k % 2) * 4 + k // 2
            lof = kk * 128 + pslot * 8
            lhsT = self.hT_hist[:, lof:lof + 8]
            for j in range(G):
                wof = k * H3 + j * 768
                nc.tensor.matmul(gates[32 * j:32 * j + 8, 0:512], lhsT,
                                 self.w_sb[:, wof:wof + 512],
                                 start=False, stop=False,
                                 tile_position=(0, 32 * j))
                nc.tensor.matmul(gates[32 * j:32 * j + 8, 512:768], lhsT,
                                 self.w_sb[:, wof + 512:wof + 768],
                                 start=False, stop=(k == KD - 1),
                                 tile_position=(0, 32 * j))

        grz = pool.tile([128, 512], F32, name="grz")
        nc.vector.tensor_add(grz[:NP, 0:256], gates[:NP, 0:256],
                             xgt[:NP, 0:256])
        nc.vector.scalar_tensor_tensor(
            grz[:NP, 256:512], gates[:NP, 256:512],
            self.zpad[:NP, ds(iv * U + u + off, 1)], xgt[:NP, 256:512],
            op0=ALU.add, op1=ALU.add)
        rz = pool.tile([128, 512], F32, name="rz")
        nc.scalar.activation(rz[:NP], grz[:NP], AF.Sigmoid)
        t2a = pool.tile([128, 256], F32, name="t2a")
        nc.vector.tensor_add(t2a[:NP], gates[:NP, 512:768], self.bhn[:NP])
        t2 = pool.tile([128, 256], F32, name="t2")
        nc.vector.tensor_mul(t2[:NP], rz[:NP, 0:256], t2a[:NP])
        npre = pool.tile([128, 256], F32, name="npre")
        nc.vector.tensor_add(npre[:NP], t2[:NP], xgt[:NP, 512:768])
        nn = pool.tile([128, 256], F32, name="nn")
        nc.scalar.activation(nn[:NP], npre[:NP], AF.Tanh)
        dlt = pool.tile([128, 256], F32, name="dlt")
        nc.vector.tensor_sub(dlt[:NP], self.hgrp[:NP], nn[:NP])
        e = pool.tile([128, 256], F32, name="e")
        nc.vector.tensor_mul(e[:NP], rz[:NP, 256:512], dlt[:NP])
        nc.vector.tensor_add(self.hgrp[:NP], nn[:NP], e[:NP])

        tp = ppt.tile([128, 256], F32, name="tp")
        for cc in range(2):
            nc.tensor.transpose(tp[:, 128 * cc:128 * cc + NP],
                                self.hgrp[0:NP, 128 * cc:128 * (cc + 1)],
                                ident[0:NP, 0:NP])
        tp4 = tp.rearrange("p (c j r) -> p (c j) r", c=2, j=G)[:, :, 0:B]
        hist4 = self.hT_hist.rearrange("p (kk s b) -> p kk s b",
                                       kk=8, s=U)
        nc.scalar.activation(hist4[:, :, slot, :], tp4, AF.Copy)

    def flush(self, iv):
        """Flush h.T for scan steps s = flush_lo + iv*U + [0,U) to
        hT_out t-idx (s-flush_lo) ascending, or n_out-1-(s-flush_lo)
        descending when flush_rev."""
        nc = self.tc.nc
        U = self.U
        UB = U * B
        for kk in range(KD):
            k = (kk % 4) * 2 + kk // 4
            src = self.hT_hist[:, kk * 128:(kk + 1) * 128]
            if self.flush_rev:
                dst = self.hT_out[k * 128:(k + 1) * 128,
                                  ds(iv * (-UB) + (self.n_out - U) * B, UB)]
            else:
                dst = self.hT_out[k * 128:(k + 1) * 128, ds(iv * UB, UB)]
            nc.sync.dma_start(dst, src)


def build_scan_pair(tc, specs, zeros_st, zrhs, ident, zeros_bf):
    nc = tc.nc
    U = 16
    with contextlib.ExitStack() as c:
        streams = [ScanStream(tc, sp["name"], c, sp["w"], sp["bhn"],
                              sp["zpad"], sp["xg"], sp["hT"],
                              sp["rev_base"], sp["flush_rev"],
                              sp["n_steps"], sp["flush_lo"], zeros_bf, U=U)
                   for sp in specs]
        n_steps = specs[0]["n_steps"]
        flush_lo = specs[0]["flush_lo"]
        assert all(sp["n_steps"] == n_steps and sp["flush_lo"] == flush_lo
                   for sp in specs)
        assert flush_lo % U == 0 and n_steps % U == 0
        nf = flush_lo // U
        if nf > 0:
            with tc.For_i(0, nf) as iv:
                for u in range(U):
                    for s in streams:
                        s.step(iv, u, 0, zeros_st, zrhs, ident)
        with tc.For_i(0, (n_steps - flush_lo) // U) as iv:
            for u in range(U):
                for s in streams:
                    s.step(iv, u, flush_lo, zeros_st, zrhs, ident)
            for s in streams:
                s.flush(iv)


def build_proj(tc, dram, zeros_st, zrhs, ident, nt):
    """x2 = x_own + concat(h1A,h1B) @ gru_out.T; x2nT (bf16) for FFN."""
    nc = tc.nc
    own0 = 2 * W * B
    LB = L * B
    with contextlib.ExitStack() as c:
        wp = c.enter_context(tc.tile_pool(name="pj_w", bufs=1))
        pool = c.enter_context(tc.tile_pool(name="pj_t", bufs=3))
        pp = c.enter_context(tc.tile_pool(name="pj_p", bufs=4, space="PSUM"))

        gw = wp.tile([128, 2 * KD * D], BF16, name="gw")
        for k in range(2 * KD):
            nc.sync.dma_start(gw[:, k * D:(k + 1) * D],
                              dram["gru_wT"][k * 128:(k + 1) * 128, :])
        h1sb = wp.tile([128, 2 * KD * LB], BF16, name="h1sb")
        for k in range(2 * KD):
            srcv = dram["hT1_A"] if k < KD else dram["hT1_B"]
            kk = k % KD
            nc.sync.dma_start(h1sb[:, k * LB:(k + 1) * LB],
                              srcv[kk * 128:(kk + 1) * 128, :])

        for tv in range(nt):
            tok = tv * 128
            x2 = pool.tile([128, D], F32, name="x2")
            for cc in range(2):
                ps = pp.tile([128, 512], F32, name="ps")
                nc.tensor.matmul(ps[:], zeros_st[:], zrhs[:],
                                 start=True, stop=False)
                for k in range(2 * KD):
                    nc.tensor.matmul(
                        ps[:], h1sb[:, k * LB + tok:k * LB + tok + 128],
                        gw[:, k * D + 512 * cc:k * D + 512 * cc + 512],
                        start=False, stop=(k == 2 * KD - 1))
                xt = pool.tile([128, 512], F32, name="xt")
                nc.sync.dma_start(
                    xt[:], dram["x_win"][own0 + tok:own0 + tok + 128,
                                         512 * cc:512 * cc + 512])
                nc.vector.tensor_add(x2[:, 512 * cc:512 * cc + 512],
                                     ps[:], xt[:])
            nc.sync.dma_start(dram["x2"][tok:tok + 128, :], x2[:])
            sq = pool.tile([128, D], F32, name="sq")
            ssum = pool.tile([128, 1], F32, name="ssum")
            nc.scalar.activation(sq[:], x2[:], AF.Square, accum_out=ssum[:])
            m = pool.tile([128, 1], F32, name="m")
            nc.vector.tensor_scalar(m[:], ssum[:], 1.0 / D, EPS,
                                    op0=ALU.mult, op1=ALU.add)
            r = pool.tile([128, 1], F32, name="r")
            nc.vector.reciprocal(r[:], m[:])
            s2 = pool.tile([128, 1], F32, name="s2")
            nc.scalar.activation(s2[:], r[:], AF.Sqrt)
            x2n = pool.tile([128, D], F32, name="x2n")
            nc.vector.tensor_scalar_mul(x2n[:], x2[:], s2[:])
            for k in range(KD):
                tpp = pp.tile([128, 128], F32, name="tpp")
                nc.tensor.transpose(tpp[:], x2n[:, k * 128:(k + 1) * 128],
                                    ident[:])
                xc = pool.tile([128, 128], BF16, name="xc")
                nc.scalar.activation(xc[:], tpp[:], AF.Copy)
                nc.sync.dma_start(
                    dram["x2nT"][k * 128:(k + 1) * 128, tok:tok + 128],
                    xc[:])


def build_ffn13(tc, dram, zeros_st, zrhs, ident, nt):
    nc = tc.nc
    LB = L * B
    with contextlib.ExitStack() as c:
        wp = c.enter_context(tc.tile_pool(name="fb_w", bufs=1))
        pool = c.enter_context(tc.tile_pool(name="fb_t", bufs=3))
        pp = c.enter_context(tc.tile_pool(name="fb_p", bufs=2, space="PSUM"))

        w1 = wp.tile([128, KD * FFN], BF16, name="w1")
        w3 = wp.tile([128, KD * FFN], BF16, name="w3")
        for k in range(KD):
            nc.sync.dma_start(w1[:, k * FFN:(k + 1) * FFN],
                              dram["w1T"][k * 128:(k + 1) * 128, :])
            nc.sync.dma_start(w3[:, k * FFN:(k + 1) * FFN],
                              dram["w3T"][k * 128:(k + 1) * 128, :])
        xsb = wp.tile([128, KD * LB], BF16, name="xsb")
        for k in range(KD):
            nc.sync.dma_start(xsb[:, k * LB:(k + 1) * LB],
                              dram["x2nT"][k * 128:(k + 1) * 128, :])

        FCH = [(c0, min(512, FFN - c0)) for c0 in range(0, FFN, 512)]
        for tv in range(nt):
            tok = tv * 128
            for (c0, cn) in FCH:
                p1 = pp.tile([128, 512], F32, name="p1")
                p3 = pp.tile([128, 512], F32, name="p3")
                nc.tensor.matmul(p1[:, :cn], zeros_st[:], zrhs[:, :cn],
                                 start=True, stop=False)
                nc.tensor.matmul(p3[:, :cn], zeros_st[:], zrhs[:, :cn],
                                 start=True, stop=False)
                for k in range(KD):
                    st = xsb[:, k * LB + tok:k * LB + tok + 128]
                    nc.tensor.matmul(p1[:, :cn], st,
                                     w1[:, k * FFN + c0:k * FFN + c0 + cn],
                                     start=False, stop=(k == KD - 1))
                    nc.tensor.matmul(p3[:, :cn], st,
                                     w3[:, k * FFN + c0:k * FFN + c0 + cn],
                                     start=False, stop=(k == KD - 1))
                sl = pool.tile([128, 512], F32, name="sl")
                nc.scalar.activation(sl[:, :cn], p1[:, :cn], AF.Silu)
                h1c = pool.tile([128, 512], F32, name="h1c")
                nc.vector.tensor_mul(h1c[:, :cn], sl[:, :cn], p3[:, :cn])
                for q in range(cn // 128):
                    tpp = pp.tile([128, 128], F32, name="tpp")
                    nc.tensor.transpose(
                        tpp[:], h1c[:, q * 128:(q + 1) * 128], ident[:])
                    hc = pool.tile([128, 128], BF16, name="hc")
                    nc.scalar.activation(hc[:], tpp[:], AF.Copy)
                    kf = (c0 + q * 128) // 128
                    nc.sync.dma_start(
                        dram["h1T"][kf * 128:(kf + 1) * 128, tok:tok + 128],
                        hc[:])


def build_ffn2(tc, dram, zeros_st, zrhs, nt):
    nc = tc.nc
    LB = L * B
    with contextlib.ExitStack() as c:
        wp = c.enter_context(tc.tile_pool(name="fc_w", bufs=1))
        pool = c.enter_context(tc.tile_pool(name="fc_t", bufs=3))
        pp = c.enter_context(tc.tile_pool(name="fc_p", bufs=4, space="PSUM"))

        w2 = wp.tile([128, KF * D], BF16, name="w2")
        for k in range(KF):
            nc.sync.dma_start(w2[:, k * D:(k + 1) * D],
                              dram["w2T"][k * 128:(k + 1) * 128, :])
        hsb = wp.tile([128, KF * LB], BF16, name="hsb")
        for k in range(KF):
            nc.sync.dma_start(hsb[:, k * LB:(k + 1) * LB],
                              dram["h1T"][k * 128:(k + 1) * 128, :])

        for tv in range(nt):
            tok = tv * 128
            for cc in range(2):
                ps = pp.tile([128, 512], F32, name="ps")
                nc.tensor.matmul(ps[:], zeros_st[:], zrhs[:],
                                 start=True, stop=False)
                for k in range(KF):
                    nc.tensor.matmul(
                        ps[:], hsb[:, k * LB + tok:k * LB + tok + 128],
                        w2[:, k * D + 512 * cc:k * D + 512 * cc + 512],
                        start=False, stop=(k == KF - 1))
                xt = pool.tile([128, 512], F32, name="xt")
                nc.sync.dma_start(
                    xt[:], dram["x2"][tok:tok + 128,
                                      512 * cc:512 * cc + 512])
                yo = pool.tile([128, 512], F32, name="yo")
                nc.vector.tensor_add(yo[:], ps[:], xt[:])
                nc.sync.dma_start(
                    dram["y"][tok:tok + 128, 512 * cc:512 * cc + 512],
                    yo[:])


def build_program(nc):
    dram = {}

    def din(name, shape, dt=F32R):
        dram[name] = nc.dram_tensor(name, shape, dt, kind="ExternalInput").ap()

    def dout(name, shape, dt=F32):
        dram[name] = nc.dram_tensor(name, shape, dt,
                                    kind="ExternalOutput").ap()

    def dtmp(name, shape, dt=F32R):
        dram[name] = nc.dram_tensor(name, shape, dt).ap()

    din("x_win", [T0X * B, D], F32)
    din("x_winT", [D, T0X * B], BF16)
    for ss in ("A", "B"):
        din(f"wA_{ss}", [D, H3], BF16)
        din(f"biasA_{ss}", [128, H3], F32)
        din(f"wD_{ss}", [2 * D, H3], BF16)
        din(f"biasD_{ss}", [128, H3], F32)
        din(f"wS0_{ss}", [128, KD * H3], BF16)
        din(f"bhn0_{ss}", [128, 256], F32)
        din(f"wS1_{ss}", [128, KD * H3], BF16)
        din(f"bhn1_{ss}", [128, 256], F32)
        din(f"zpad0_{ss}", [128, T0], F32)
        din(f"zpad1_{ss}", [128, T1], F32)
    din("zeros", [128, 1024])
    din("zeros_bf", [128, 1024], BF16)
    din("gru_wT", [2 * D, D], BF16)
    din("w1T", [D, FFN], BF16)
    din("w3T", [D, FFN], BF16)
    din("w2T", [FFN, D], BF16)
    dout("y", [L * B, D])

    for ss in ("A", "B"):
        dtmp(f"xg0_{ss}", [T0X // 16, G, B, 16, 768], BF16)
        dtmp(f"xg1_{ss}", [T1 // 16, G, B, 16, 768], BF16)
        dtmp(f"hT0_{ss}", [D, T0H * B], BF16)
        dtmp(f"hT1_{ss}", [D, L * B], BF16)
    dtmp("x2", [L * B, D], F32)
    dtmp("x2nT", [D, L * B], BF16)
    dtmp("h1T", [FFN, L * B], BF16)

    NT0 = T0X * B // 128       # 16
    NT1 = T1 * B // 128        # 10
    NTP = L * B // 128         # 8

    with tile.TileContext(nc) as tc:
        with tc.tile_pool(name="consts", bufs=1) as consts:
            zeros_st = consts.tile([1, 128], F32R, name="zeros_st")
            nc.sync.dma_start(zeros_st[:], dram["zeros"][0:1, 0:128])
            zrhs = consts.tile([1, 512], F32R, name="zrhs")
            nc.sync.dma_start(zrhs[:], dram["zeros"][0:1, 0:512])
            ident = consts.tile([128, 128], F32, name="ident")
            make_identity(nc, ident[:])
            s_sb = consts.tile([128, NT0], F32, name="s_sb")

            with nc.named_scope("norm"):
                build_norm_stats(tc, dram["x_win"], s_sb, NT0)
            xt_views = [dram["x_winT"][k * 128:(k + 1) * 128, :]
                        for k in range(KD)]
            with nc.named_scope("xg0"):
                build_xg_gemm(tc, xt_views, KD,
                              [dram["wA_A"], dram["wA_B"]],
                              [dram["biasA_A"], dram["biasA_B"]], s_sb,
                              [dram["xg0_A"], dram["xg0_B"]],
                              zeros_st, zrhs, NT0)
            with nc.named_scope("scan0"):
                build_scan_pair(
                    tc,
                    [dict(name="s0A", w=dram["wS0_A"], bhn=dram["bhn0_A"],
                          zpad=dram["zpad0_A"], xg=dram["xg0_A"],
                          hT=dram["hT0_A"], rev_base=None, flush_rev=False,
                          n_steps=T0, flush_lo=W),
                     dict(name="s0B", w=dram["wS0_B"], bhn=dram["bhn0_B"],
                          zpad=dram["zpad0_B"], xg=dram["xg0_B"],
                          hT=dram["hT0_B"], rev_base=T0X - 1,
                          flush_rev=True, n_steps=T0, flush_lo=W)],
                    zeros_st, zrhs, ident, dram["zeros_bf"])
            # xg1_A covers h0 idx [0,T1) ascending; xg1_B covers idx
            # [W,T0H) ascending (its scan fetches it reversed).
            h0a = dram["hT0_A"]
            h0b = dram["hT0_B"]
            for ss, off in (("A", 0), ("B", W * B)):
                sv = [h0a[k * 128:(k + 1) * 128, off:off + T1 * B]
                      for k in range(KD)]
                sv += [h0b[k * 128:(k + 1) * 128, off:off + T1 * B]
                       for k in range(KD)]
                with nc.named_scope(f"xg1{ss}"):
                    build_xg_gemm(tc, sv, 2 * KD, [dram[f"wD_{ss}"]],
                                  [dram[f"biasD_{ss}"]], None,
                                  [dram[f"xg1_{ss}"]],
                                  zeros_st, zrhs, NT1, wdt=BF16, sdt=BF16)
            with nc.named_scope("scan1"):
                build_scan_pair(
                    tc,
                    [dict(name="s1A", w=dram["wS1_A"], bhn=dram["bhn1_A"],
                          zpad=dram["zpad1_A"], xg=dram["xg1_A"],
                          hT=dram["hT1_A"], rev_base=None, flush_rev=False,
                          n_steps=T1, flush_lo=W),
                     dict(name="s1B", w=dram["wS1_B"], bhn=dram["bhn1_B"],
                          zpad=dram["zpad1_B"], xg=dram["xg1_B"],
                          hT=dram["hT1_B"], rev_base=T1 - 1,
                          flush_rev=True, n_steps=T1, flush_lo=W)],
                    zeros_st, zrhs, ident, dram["zeros_bf"])
            with nc.named_scope("proj"):
                build_proj(tc, dram, zeros_st, zrhs, ident, NTP)
            with nc.named_scope("ffn13"):
                build_ffn13(tc, dram, zeros_st, zrhs, ident, NTP)
            with nc.named_scope("ffn2"):
                build_ffn2(tc, dram, zeros_st, zrhs, NTP)
    return dram


# ================================================================== driver
_CACHE = {}


def _prep_shared(inputs):
    import ml_dtypes
    bf = ml_dtypes.bfloat16
    gnw = np.asarray(inputs["gru_norm_w"], np.float32)
    fnw = np.asarray(inputs["ffn_norm_w"], np.float32)
    sh = {}
    for di, ss in ((0, "A"), (1, "B")):
        sh[f"wA_{ss}"] = prep_gemm_weights(
            np.asarray(inputs["w_ih_l0"], np.float32)[di], gnw).astype(bf)
        sh[f"biasA_{ss}"] = prep_gemm_bias(
            np.asarray(inputs["b_ih_l0"], np.float32)[di],
            np.asarray(inputs["b_hh_l0"], np.float32)[di])
        sh[f"wD_{ss}"] = prep_gemm_weights(
            np.asarray(inputs["w_ih_l1"], np.float32)[di]).astype(bf)
        sh[f"biasD_{ss}"] = prep_gemm_bias(
            np.asarray(inputs["b_ih_l1"], np.float32)[di],
            np.asarray(inputs["b_hh_l1"], np.float32)[di])
        for Lx in (0, 1):
            sh[f"wS{Lx}_{ss}"] = prep_scan_weights(
                np.asarray(inputs[f"w_hh_l{Lx}"], np.float32)[di]).astype(bf)
            sh[f"bhn{Lx}_{ss}"] = prep_bhn_scan(
                np.asarray(inputs[f"b_hh_l{Lx}"], np.float32)[di])
    sh["zeros"] = np.zeros((128, 1024), np.float32)
    sh["zeros_bf"] = np.zeros((128, 1024), bf)
    sh["gru_wT"] = np.ascontiguousarray(
        np.asarray(inputs["gru_out_w"], np.float32).T).astype(bf)
    sh["w1T"] = np.ascontiguousarray(
        (np.asarray(inputs["w1"], np.float32) * fnw[None, :]).T).astype(bf)
    sh["w3T"] = np.ascontiguousarray(
        (np.asarray(inputs["w3"], np.float32) * fnw[None, :]).T).astype(bf)
    sh["w2T"] = np.ascontiguousarray(
        np.asarray(inputs["w2"], np.float32).T).astype(bf)
    return sh


def _host_inputs_spmd(inputs):
    x = np.asarray(inputs["x"], np.float32)          # [B, S, D]
    sh = _prep_shared(inputs)
    im_list = []
    for c in range(NCORE):
        im = dict(sh)
        lo = c * L - 2 * W
        idx = np.arange(lo, lo + T0X)
        valid = (idx >= 0) & (idx < S)
        xw = np.zeros((T0X, B, D), np.float32)
        xw[valid] = x[:, idx[valid], :].transpose(1, 0, 2)
        xw = xw.reshape(T0X * B, D)
        im["x_win"] = np.ascontiguousarray(xw)
        import ml_dtypes
        im["x_winT"] = np.ascontiguousarray(xw.T).astype(ml_dtypes.bfloat16)
        for ss, rev in (("A", False), ("B", True)):
            sidx = np.arange(T0)
            widx = sidx if not rev else (T0X - 1 - sidx)
            t = lo + widx
            z0 = np.zeros((128, T0), np.float32)
            z0[:] = (((t < 0) | (t >= S)) * 40.0)[None, :]
            im[f"zpad0_{ss}"] = z0
            s1 = np.arange(T1)
            hidx = s1 if not rev else (T0H - 1 - s1)
            t1 = (lo + W) + hidx
            z1 = np.zeros((128, T1), np.float32)
            z1[:] = (((t1 < 0) | (t1 >= S)) * 40.0)[None, :]
            im[f"zpad1_{ss}"] = z1
        im_list.append(im)
    return im_list


def get_compiled(n_cores=NCORE):
    if "nc" not in _CACHE:
        nc = bacc.Bacc("TRN2", target_bir_lowering=False, debug=False,
                       num_devices=n_cores)
        build_program(nc)
        nc.compile()
        _CACHE["nc"] = nc
        _CACHE["n_cores"] = n_cores
    return _CACHE["nc"], _CACHE["n_cores"]


def kernel(**inputs) -> np.ndarray:
    im_list = _host_inputs_spmd(inputs)
    nc, n_cores = get_compiled()
    res = run_bass_kernel_spmd(nc, im_list, core_ids=list(range(n_cores)))
    y = np.zeros((B, S, D), np.float32)
    for c in range(n_cores):
        yc = res.results[c]["y"].reshape(L, B, D)
        y[:, c * L:(c + 1) * L, :] = yc.transpose(1, 0, 2)
    return y
